# revision 27
# baseline (speedup 1.0000x reference)
"""BrainGCN kernel for 8 Trainium2 NeuronCores (Bass/Tile).

Strategy (v2):
- Nodes partitioned across 8 cores (degree-sorted snake deal), shard=6272
  locals per core (49 chunks of 128). Each node is assigned to one of two
  "halves" (table windows) with a greedy per-destination balance pass so that
  every destination's in-edges split ~evenly between halves; this keeps the
  round-padded slot structure tight (~1.1x instead of 1.45x).
- conv1: host pre-expands x*dinv into per-edge-slot columns (bf16,
  feature-major, round-major within each half's chunk group). The device
  streams slabs and accumulates directly in PSUM: one [64,128] PSUM region
  per chunk, matmuls accumulate rounds (start on k==0), so no DVE adds.
  h1 = tanh(psum*dinv + b1) read out per bank.
- z2 = (h1*dinv) @ W2 per chunk (node-major), converted to bf16 and
  AllGathered per half as soon as that half's chunks finish, overlapping the
  collective with the other half's conv1 streaming. Gathered bf16 tables are
  upconverted on device to fp32 tables for dma_gather (256B rows).
- conv2: big dma_gather groups (GBLK blocks = GBLK*128 indices each) on 4
  SWDGE queues; reduction on the PE via fp32r identity-matmuls accumulating
  into per-chunk [128,64] PSUM regions (1 cycle/row), h2 read out per bank.
- FC head: per-chunk PE transposes, feature-major matmuls, fused tanh+bias.

kernel(**inputs) takes FULL inputs, preprocesses + shards on host, compiles
and runs the SPMD program on cores 0..7, and reassembles the full output.
"""

import os
import warnings

warnings.filterwarnings("ignore")

import numpy as np
import ml_dtypes

from concourse import bacc, bass, mybir, tile
from concourse.masks import make_identity
import concourse.bass_utils as bass_utils

P = 128
NCORES = 8
GQ = int(os.environ.get("GCN_GQ", "4"))  # SWDGE queues for conv2 gathers
GBLK = int(os.environ.get("GCN_GBLK", "8"))  # blocks per dma_gather
SLAB = int(os.environ.get("GCN_SLAB", "8192"))  # conv1 stream columns per DMA
PE_REDUCE = bool(int(os.environ.get("GCN_PE_REDUCE", "1")))
AG8 = bool(int(os.environ.get("GCN_AG8", "0")))  # fp8 AllGather payload
X8 = bool(int(os.environ.get("GCN_X8", "0")))  # fp8 conv1 stream
F32R = bool(int(os.environ.get("GCN_F32R", "1")))  # fp32r PE reduce pipeline
ACTDMA = bool(int(os.environ.get("GCN_ACTDMA", "1")))  # allow ACT-engine DMAs
AGBF = bool(int(os.environ.get("GCN_AGBF", "1")))  # bf16 AllGather payload + upconvert
C1PSUM = bool(int(os.environ.get("GCN_C1PSUM", "1")))  # conv1 PSUM-region accumulate
ZSCALE = bool(int(os.environ.get("GCN_ZSCALE", "1")))  # fused dinv scale on z2 copy


# ---------------------------------------------------------------------------
# Host preprocessing
# ---------------------------------------------------------------------------

def _preprocess(x, edge_index):
    """Partition nodes, build slot structures and per-core input arrays."""
    N = x.shape[0]
    E = edge_index.shape[1]
    src = np.asarray(edge_index[0], dtype=np.int64)
    dst = np.asarray(edge_index[1], dtype=np.int64)

    shard = -(-N // (NCORES * P)) * P  # 6272
    nch = shard // P  # 49
    h0ch = (nch + 1) // 2  # 25
    h1ch = nch - h0ch  # 24
    H0 = h0ch * P  # 3200 positions per core in half 0
    H1 = h1ch * P  # 3072

    deg = 1 + np.bincount(dst, minlength=N)  # includes self-loop
    dinv = (1.0 / np.sqrt(deg)).astype(np.float32)

    counts = np.array([N // NCORES + (c < N % NCORES) for c in range(NCORES)])
    # phase A: global degree sort (desc), snake deal to cores
    order = np.argsort(-deg, kind="stable")
    core_of = np.empty(N, np.int32)
    taken = np.zeros(NCORES, np.int64)
    ci = 0
    direction = 1
    for v in order:
        for _ in range(NCORES):
            if taken[ci] < counts[ci]:
                break
            ci = (ci + direction) % NCORES
        core_of[v] = ci
        taken[ci] += 1
        ci += direction
        if ci == NCORES:
            ci, direction = NCORES - 1, -1
        elif ci == -1:
            ci, direction = 0, 1

    # out-adjacency (dests per source), for the balance pass
    sorder = np.argsort(src, kind="stable")
    src_ss = src[sorder]
    dst_ss = dst[sorder]
    ostarts = np.searchsorted(src_ss, np.arange(N))
    oends = np.searchsorted(src_ss, np.arange(N) + 1)

    # greedy balanced half assignment: process nodes in degree-desc order;
    # put v in the half where its out-destinations currently have fewer
    # sources, respecting per-(core,half) capacity (one pad row reserved).
    d0 = np.zeros(N, np.int32)
    d1 = np.zeros(N, np.int32)
    n0 = np.zeros(NCORES, np.int64)
    n1 = np.zeros(NCORES, np.int64)
    cap0, cap1 = H0 - 1, H1 - 1
    half_of = np.empty(N, np.int8)
    for v in order:
        c = core_of[v]
        D = dst_ss[ostarts[v]:oends[v]]
        s0 = int(d0[D].sum()) + int(d0[v])  # self-loop dest is v itself
        s1 = int(d1[D].sum()) + int(d1[v])
        h = 0 if s0 <= s1 else 1
        if h == 0 and n0[c] >= cap0:
            h = 1
        elif h == 1 and n1[c] >= cap1:
            h = 0
        half_of[v] = h
        if h == 0:
            np.add.at(d0, D, 1)
            d0[v] += 1
            n0[c] += 1
        else:
            np.add.at(d1, D, 1)
            d1[v] += 1
            n1[c] += 1
    assert (n0 <= cap0).all() and (n1 <= cap1).all()

    # refinement sweeps: re-assign each node to the half that best balances
    # its destinations' source counts (capacity-respecting)
    for _ in range(3):
        moved = 0
        for v in order:
            c = core_of[v]
            D = dst_ss[ostarts[v]:oends[v]]
            h_cur = half_of[v]
            # counts with v removed
            if h_cur == 0:
                np.add.at(d0, D, -1)
                d0[v] -= 1
                n0[c] -= 1
            else:
                np.add.at(d1, D, -1)
                d1[v] -= 1
                n1[c] -= 1
            s0 = int(d0[D].sum()) + int(d0[v])
            s1 = int(d1[D].sum()) + int(d1[v])
            h = 0 if s0 <= s1 else 1
            if h == 0 and n0[c] >= cap0:
                h = 1
            elif h == 1 and n1[c] >= cap1:
                h = 0
            if h != h_cur:
                moved += 1
            half_of[v] = h
            if h == 0:
                np.add.at(d0, D, 1)
                d0[v] += 1
                n0[c] += 1
            else:
                np.add.at(d1, D, 1)
                d1[v] += 1
                n1[c] += 1
        if moved == 0:
            break

    # phase B: positions within each (core, half) by total degree desc
    pos_of = np.full(N, -1, np.int64)
    for c in range(NCORES):
        for h, base in ((0, 0), (1, H0)):
            mask = (core_of == c) & (half_of == h)
            mem = np.nonzero(mask)[0]
            key = np.lexsort((-(d0[mem] if h == 0 else d1[mem]), -deg[mem]))
            pos_of[mem[key]] = base + np.arange(len(mem))

    # global half-table rows
    grow_h = np.where(
        half_of == 0,
        core_of.astype(np.int64) * H0 + pos_of,
        core_of.astype(np.int64) * H1 + (pos_of - H0),
    )

    # per-node in-edge src lists
    eorder = np.argsort(dst, kind="stable")
    dst_s = dst[eorder]
    src_s = src[eorder]
    starts = np.searchsorted(dst_s, np.arange(N))
    ends = np.searchsorted(dst_s, np.arange(N) + 1)

    # round counts per chunk (global max over cores)
    # conv1: total degree; conv2: per-source-half degree of the dest
    Ktg = np.zeros(nch, np.int32)
    K0g = np.zeros(nch, np.int32)
    K1g = np.zeros(nch, np.int32)
    ch_of = (pos_of // P).astype(np.int32)
    np.maximum.at(Ktg, ch_of, deg.astype(np.int32))
    np.maximum.at(K0g, ch_of, d0)
    np.maximum.at(K1g, ch_of, d1)
    assert (K0g >= 1).all() and (K1g >= 1).all() and (Ktg >= 1).all()

    # conv1 block list: half-group-major, then round-major
    def round_major(Karr, chs):
        kmax = int(Karr[chs].max()) if len(chs) else 0
        blocks = []
        for k in range(kmax):
            for ch in chs:
                if Karr[ch] > k:
                    blocks.append((k, ch))
        return blocks

    blocks1_g0 = round_major(Ktg, list(range(h0ch)))
    blocks1_g1 = round_major(Ktg, list(range(h0ch, nch)))
    blocks1 = blocks1_g0 + blocks1_g1
    nb1_g0 = len(blocks1_g0)
    S1 = len(blocks1) * P

    # conv2 block lists per source half, round-major over all chunks
    blocks2 = {
        0: round_major(K0g, list(range(nch))),
        1: round_major(K1g, list(range(nch))),
    }

    # conv2 gather groups (<= GBLK blocks each, within one source half)
    groups = []
    for half in (0, 1):
        blks = blocks2[half]
        for i in range(0, len(blks), GBLK):
            groups.append((half, blks[i : i + GBLK]))

    tot2 = (len(blocks2[0]) + len(blocks2[1])) * P
    per_core_work = (E + N) / NCORES
    print(
        f"[pre] shard={shard} conv1 slots={S1} ({S1/per_core_work:.3f}x) "
        f"conv2 slots={tot2} ({tot2/per_core_work:.3f}x) groups={len(groups)}"
    )

    node_at = np.full((NCORES, shard), -1, np.int64)
    node_at[core_of, pos_of] = np.arange(N)

    # PE reduce segments for conv2: runs of consecutive ch at the same k
    # within one PSUM bank. PSUM accumulation groups are bank-granular
    # (ZERO_REGION=2KB): exactly one start (first segment touching the bank,
    # which zeroes the whole bank) and one stop (last segment) per bank.
    group_segs = []
    for half, blks in groups:
        segs = []
        i = 0
        while i < len(blks):
            k, ch = blks[i]
            r = 1
            while (
                i + r < len(blks)
                and blks[i + r] == (k, ch + r)
                and (ch + r) // 8 == ch // 8
            ):
                r += 1
            segs.append([i, ch // 8, (ch % 8) * 64, r, False, False])
            i += r
        group_segs.append(segs)
    seen_bank = set()
    last_seg_of_bank = {}
    for gi, segs in enumerate(group_segs):
        for si_, seg in enumerate(segs):
            b = seg[1]
            if b not in seen_bank:
                seen_bank.add(b)
                seg[4] = True
            last_seg_of_bank[b] = (gi, si_)
    for b, (gi, si_) in last_seg_of_bank.items():
        group_segs[gi][si_][5] = True

    # conv1 PE segments: per 512-col matmul window of each slab we need runs
    # of consecutive ch within one conv1 PSUM bank (4 regions of [64,128]).
    # Built on the fly in _build from blocks1.

    xdt_np = (
        ml_dtypes.float8_e4m3fn
        if os.environ.get("GCN_X8", "0") == "1"
        else ml_dtypes.bfloat16
    )
    xs = x.astype(np.float32) * dinv[:, None]
    xsT = np.ascontiguousarray(xs.T).astype(xdt_np)  # [128, N]

    # per-core slot source arrays
    b1_of = {}
    for i, (k, ch) in enumerate(blocks1):
        b1_of[(k, ch)] = i
    b2_of = {0: {}, 1: {}}
    for h in (0, 1):
        for i, (k, ch) in enumerate(blocks2[h]):
            b2_of[h][(k, ch)] = i

    per_core = []
    for c in range(NCORES):
        src1 = np.full((len(blocks1), P), -1, np.int64)
        idx2 = {
            0: np.full((len(blocks2[0]), P), -1, np.int64),
            1: np.full((len(blocks2[1]), P), -1, np.int64),
        }
        zero_row = {0: c * H0 + H0 - 1, 1: c * H1 + H1 - 1}
        for pos in range(shard):
            v = node_at[c, pos]
            if v < 0:
                continue
            ch, p = pos // P, pos % P
            e0 = src_s[starts[v] : ends[v]]
            halves = half_of[e0]
            l0 = e0[halves == 0]
            l1 = e0[halves == 1]
            if half_of[v] == 0:
                l0 = np.concatenate(([v], l0))
            else:
                l1 = np.concatenate(([v], l1))
            ltot = np.concatenate((l0, l1))
            for k in range(len(ltot)):
                src1[b1_of[(k, ch)], p] = ltot[k]
            for h, lh in ((0, l0), (1, l1)):
                for k in range(len(lh)):
                    idx2[h][b2_of[h][(k, ch)], p] = grow_h[lh[k]]

        flat1 = src1.reshape(-1)
        x_exp = np.zeros((P, S1), dtype=xdt_np)
        valid = flat1 >= 0
        x_exp[:, valid] = xsT[:, flat1[valid]]

        slabs = []
        for gi, (half, blks) in enumerate(groups):
            idxs = np.empty((len(blks), P), np.int64)
            for j, (k, ch) in enumerate(blks):
                row = idx2[half][b2_of[half][(k, ch)]]
                idxs[j] = np.where(row >= 0, row, zero_row[half])
            flat = idxs.reshape(-1)
            assert flat.max() < 32768, flat.max()
            S = len(flat) // 16
            wrapped = flat.reshape(S, 16).T.astype(np.int16)  # [16, S]
            slabs.append(wrapped)
        idx_cat = np.concatenate(slabs, axis=1)
        idx_rep = np.tile(idx_cat, (8, 1))  # [128, sum S]

        dinv_loc = np.zeros(shard, np.float32)
        valid_pos = node_at[c] >= 0
        dinv_loc[valid_pos] = dinv[node_at[c][valid_pos]]
        dinv_fm = np.tile(dinv_loc[None, :], (64, 1)).astype(ml_dtypes.bfloat16)
        dinv_nm = dinv_loc.reshape(nch, P).T.astype(np.float32).copy()  # [128,nch]

        per_core.append(
            dict(x_exp=x_exp, idx=idx_rep, dinv_fm=dinv_fm, dinv_nm=dinv_nm)
        )

    struct = dict(
        N=N,
        shard=shard,
        nch=nch,
        h0ch=h0ch,
        h1ch=h1ch,
        H0=H0,
        H1=H1,
        blocks1=blocks1,
        nb1_g0=nb1_g0,
        groups=groups,
        group_segs=group_segs,
        blocks2=blocks2,
        S1=S1,
        node_at=node_at,
        idx_cols=per_core[0]["idx"].shape[1],
        K1g=K1g,
    )
    return struct, per_core, dinv


# ---------------------------------------------------------------------------
# Program builder
# ---------------------------------------------------------------------------

def _build(st, weights, n_passes=1):
    """Build the SPMD Bass program."""
    shard, nch = st["shard"], st["nch"]
    S1 = st["S1"]
    blocks1 = st["blocks1"]
    nb1_g0 = st["nb1_g0"]
    groups = st["groups"]
    group_segs = st["group_segs"]
    H0, H1 = st["H0"], st["H1"]
    h0ch, h1ch = st["h0ch"], st["h1ch"]
    bf16 = mybir.dt.bfloat16
    f32 = mybir.dt.float32
    f32r = mybir.dt.float32r
    tdt = f32r if (F32R and PE_REDUCE) else f32
    agdt = (mybir.dt.float8e4 if AG8 else bf16) if AGBF else tdt
    xdt = mybir.dt.float8e4 if X8 else bf16

    fb2 = float(np.asarray(weights["fc_b2"]).reshape(-1)[0])

    nc = bacc.Bacc(
        "TRN2",
        target_bir_lowering=False,
        debug=False,
        enable_asserts=False,
        num_devices=NCORES,
        num_swdge_queues=GQ,
    )

    x_exp_in = nc.dram_tensor("x_exp", [P, S1], xdt, kind="ExternalInput")
    idx_in = nc.dram_tensor(
        "idx2", [P, st["idx_cols"]], mybir.dt.int16, kind="ExternalInput"
    )
    dinv_fm_in = nc.dram_tensor("dinv_fm", [64, shard], bf16, kind="ExternalInput")
    dinv_nm_in = nc.dram_tensor("dinv_nm", [P, nch], f32, kind="ExternalInput")
    w1_in = nc.dram_tensor("w1", [P, 64], xdt, kind="ExternalInput")
    w2_in = nc.dram_tensor("w2", [64, 64], bf16, kind="ExternalInput")
    fw1_in = nc.dram_tensor("fw1", [64, 32], bf16, kind="ExternalInput")
    fw2_in = nc.dram_tensor("fw2", [32, 1], bf16, kind="ExternalInput")
    b1_in = nc.dram_tensor("b1c", [64, 1], f32, kind="ExternalInput")
    b2e_in = nc.dram_tensor("b2e", [P, 64], f32, kind="ExternalInput")
    fb1_in = nc.dram_tensor("fb1c", [32, 1], f32, kind="ExternalInput")
    y_out = nc.dram_tensor("y", [1, shard], f32, kind="ExternalOutput")

    with tile.TileContext(nc) as tc:
        with (
            tc.tile_pool(name="const", bufs=1) as constp,
            tc.tile_pool(name="big", bufs=1) as bigp,
            tc.tile_pool(name="xslab", bufs=3 if C1PSUM else 2) as xslabp,
            tc.tile_pool(name="gstage", bufs=3) as gstagep,
            tc.tile_pool(name="upc", bufs=2) as upcp,
            tc.tile_pool(name="psum", bufs=8, space="PSUM") as psump,
            tc.tile_pool(name="small", bufs=2) as smallp,
            tc.tile_pool(name="dram", bufs=1, space="DRAM") as dramp,
        ):
            # constants
            w1_sb = constp.tile([P, 64], xdt, name="w1_sb")
            nc.sync.dma_start(out=w1_sb[:], in_=w1_in.ap())
            w2_sb = constp.tile([64, 64], bf16, name="w2_sb")
            nc.sync.dma_start(out=w2_sb[:], in_=w2_in.ap())
            fw1_sb = constp.tile([64, 32], bf16, name="fw1_sb")
            nc.sync.dma_start(out=fw1_sb[:], in_=fw1_in.ap())
            fw2_sb = constp.tile([32, 1], bf16, name="fw2_sb")
            nc.sync.dma_start(out=fw2_sb[:], in_=fw2_in.ap())
            b1_sb = constp.tile([64, 1], f32, name="b1_sb")
            nc.sync.dma_start(out=b1_sb[:], in_=b1_in.ap())
            b2e_sb = constp.tile([P, 64], f32, name="b2e_sb")
            nc.sync.dma_start(out=b2e_sb[:], in_=b2e_in.ap())
            fb1_sb = constp.tile([32, 1], f32, name="fb1_sb")
            nc.sync.dma_start(out=fb1_sb[:], in_=fb1_in.ap())
            dinv_fm = constp.tile([64, shard], bf16, name="dinv_fm_sb")
            nc.sync.dma_start(out=dinv_fm[:], in_=dinv_fm_in.ap())
            dinv_nm = constp.tile([P, nch], f32, name="dinv_nm_sb")
            nc.sync.dma_start(out=dinv_nm[:], in_=dinv_nm_in.ap())
            ident = constp.tile([P, P], f32, name="ident")
            make_identity(nc, ident[:])
            if F32R and PE_REDUCE:
                ident_r = constp.tile([P, P], f32r, name="ident_r")
                nc.vector.tensor_copy(out=ident_r[:], in_=ident[:])
            else:
                ident_r = ident
            idx_sb = constp.tile([P, st["idx_cols"]], mybir.dt.int16, name="idx_sb")
            nc.sync.dma_start(out=idx_sb[:], in_=idx_in.ap())

            # DRAM tables
            t0h = dramp.tile([NCORES * H0, 64], agdt, name="t0h", addr_space="Shared")
            t1h = dramp.tile([NCORES * H1, 64], agdt, name="t1h", addr_space="Shared")
            if AGBF:
                t0 = dramp.tile([NCORES * H0, 64], tdt, name="t0")
                t1 = dramp.tile([NCORES * H1, 64], tdt, name="t1")
            else:
                t0, t1 = t0h, t1h
            ag0_in = dramp.tile([H0, 64], agdt, name="ag0i")
            ag1_in = dramp.tile([H1, 64], agdt, name="ag1i")

            h1s = bigp.tile([64, shard], bf16, name="h1s", tag="fm")
            acc1 = None
            if not C1PSUM:
                acc1 = bigp.tile([64, shard], f32, name="acc1", tag="acc1")

            # per-chunk conv1 round counts from blocks1
            _KT = {}
            for k, ch in blocks1:
                _KT[ch] = max(_KT.get(ch, 0), k + 1)

            # ------------- conv1 (per half group) + z2 + AllGather ----------
            def conv1_half(hg):
                b_lo = 0 if hg == 0 else nb1_g0
                b_hi = nb1_g0 if hg == 0 else len(blocks1)
                ch_lo = 0 if hg == 0 else h0ch
                nch_g = h0ch if hg == 0 else h1ch
                # PSUM regions: 4 chunks of [64,128] per bank
                nbank = -(-nch_g // 4)
                regs = []
                if C1PSUM:
                    for b in range(nbank):
                        rg = psump.tile(
                            [P, 512], f32, tag="ps", bufs=8, name=f"c1r_{hg}_{b}"
                        )
                        regs.append(rg)

                c_lo, c_hi = b_lo * P, b_hi * P
                n_slabs = -(-(c_hi - c_lo) // SLAB)
                # pre-enumerate matmul segments: (slab, i, r, bank, col)
                seg_list = []
                for si in range(n_slabs):
                    s0 = c_lo + si * SLAB
                    s1_ = min(c_hi, s0 + SLAB)
                    i = s0 // P
                    bend = s1_ // P
                    while i < bend:
                        k, ch = blocks1[i]
                        lch = ch - ch_lo
                        r = 1
                        while (
                            i + r < bend
                            and blocks1[i + r] == (k, ch + r)
                            and (lch + r) // 4 == lch // 4
                        ):
                            r += 1
                        seg_list.append([si, i, r, lch // 4, (lch % 4) * P])
                        i += r
                seen_b = set()
                last_of_b = {}
                flags = []
                for j, (si, i, r, b, col) in enumerate(seg_list):
                    st_f = b not in seen_b
                    seen_b.add(b)
                    last_of_b[b] = j
                    flags.append([st_f, False])
                for b, j in last_of_b.items():
                    flags[j][1] = True

                jseg = 0
                for si in range(n_slabs):
                    s0 = c_lo + si * SLAB
                    s1_ = min(c_hi, s0 + SLAB)
                    xsl = xslabp.tile([P, SLAB], xdt, tag="xsl", name=f"xsl_{hg}_{si}")
                    eng = nc.sync if (si % 2 == 0 or not ACTDMA) else nc.scalar
                    eng.dma_start(out=xsl[:, : s1_ - s0], in_=x_exp_in.ap()[:, s0:s1_])
                    if C1PSUM:
                        while jseg < len(seg_list) and seg_list[jseg][0] == si:
                            _, i, r, b, col = seg_list[jseg]
                            st_f, sp_f = flags[jseg]
                            nc.tensor.matmul(
                                regs[b][:64, col : col + r * P],
                                lhsT=w1_sb[:],
                                rhs=xsl[:, (i * P - s0) : (i * P - s0) + r * P],
                                start=st_f,
                                stop=sp_f,
                            )
                            jseg += 1
                    else:
                        for m0 in range(s0, s1_, 512):
                            m1 = min(s1_, m0 + 512)
                            pt = psump.tile(
                                [P, 512], f32, tag="ps", bufs=8, name=f"ps1_{hg}_{m0}"
                            )
                            nc.tensor.matmul(
                                pt[:64, : m1 - m0],
                                lhsT=w1_sb[:],
                                rhs=xsl[:, m0 - s0 : m1 - s0],
                                start=True,
                                stop=True,
                            )
                            b0, bend2 = m0 // P, m1 // P
                            i = b0
                            while i < bend2:
                                k, ch = blocks1[i]
                                r = 1
                                while i + r < bend2 and blocks1[i + r] == (k, ch + r):
                                    r += 1
                                if k == 0:
                                    nc.scalar.copy(
                                        out=acc1[:, ch * P : ch * P + r * P],
                                        in_=pt[:64, (i - b0) * P : (i - b0 + r) * P],
                                    )
                                else:
                                    nc.vector.tensor_add(
                                        acc1[:, ch * P : ch * P + r * P],
                                        acc1[:, ch * P : ch * P + r * P],
                                        pt[:64, (i - b0) * P : (i - b0 + r) * P],
                                    )
                                i += r

                # h1 = tanh(acc*dinv + b1), per bank readout
                cols0 = ch_lo * P
                for b in range(nbank):
                    w = min(512, (nch_g - b * 4) * P)
                    a0 = cols0 + b * 512
                    nc.vector.tensor_mul(
                        h1s[:, a0 : a0 + w],
                        regs[b][:64, :w] if C1PSUM else acc1[:, a0 : a0 + w],
                        dinv_fm[:, a0 : a0 + w],
                    )
                gw = nch_g * P
                nc.scalar.activation(
                    h1s[:, cols0 : cols0 + gw],
                    h1s[:, cols0 : cols0 + gw],
                    mybir.ActivationFunctionType.Tanh,
                    bias=b1_sb[:, :1],
                )

                # z2 (node-major bf16) for this half's chunks
                z2st = smallp.tile(
                    [P, nch_g * 64], agdt, tag="z2st", name=f"z2st_{hg}"
                )
                for j in range(nch_g):
                    ch = ch_lo + j
                    pz = psump.tile(
                        [P, 512], f32, tag="ps", bufs=8, name=f"pz_{hg}_{j}"
                    )
                    nc.tensor.matmul(
                        pz[:, :64],
                        lhsT=h1s[:, ch * P : (ch + 1) * P],
                        rhs=w2_sb[:],
                        start=True,
                        stop=True,
                    )
                    # z2 = (h1 @ W2) * dinv (per-node row scale, fused here)
                    if not ZSCALE:
                        nc.vector.tensor_scalar_mul(
                            z2st[:, j * 64 : (j + 1) * 64],
                            pz[:, :64],
                            dinv_nm[:, ch : ch + 1],
                        )
                    elif j % 2 == 0:
                        nc.scalar.activation(
                            z2st[:, j * 64 : (j + 1) * 64],
                            pz[:, :64],
                            mybir.ActivationFunctionType.Copy,
                            scale=dinv_nm[:, ch : ch + 1],
                        )
                    else:
                        nc.vector.tensor_scalar_mul(
                            z2st[:, j * 64 : (j + 1) * 64],
                            pz[:, :64],
                            dinv_nm[:, ch : ch + 1],
                        )

                ag_in = ag0_in if hg == 0 else ag1_in
                th = t0h if hg == 0 else t1h
                (nc.scalar if ACTDMA else nc.sync).dma_start(
                    out=ag_in[:].rearrange("(c p) f -> p c f", p=P),
                    in_=z2st[:].rearrange("p (c f) -> p c f", f=64),
                )
                nc.gpsimd.collective_compute(
                    "AllGather",
                    mybir.AluOpType.bypass,
                    replica_groups=[list(range(NCORES))],
                    ins=[ag_in.opt()],
                    outs=[th.opt()],
                )

            # upconvert one bf16 half-table to fp32, in pieces.
            # partition-major view: partition p holds table rows
            # [p*a_tot, (p+1)*a_tot) so each DMA is 128 contiguous runs.
            def upconvert(th, tf, rows, who, act_ok, gate=None):
                # act_ok=False keeps the ACT queue clear (so a later ag DMA
                # is not stuck behind pieces that wait on this AllGather).
                # gate: tiny DRAM AP whose write must precede this phase —
                # read one element into the staging tiles so the scheduler
                # cannot hoist these pieces ahead of the gate's producer.
                a_tot = rows // P
                PIECE = -(-a_tot // 8)
                for pi, a0 in enumerate(range(0, a_tot, PIECE)):
                    a1 = min(a_tot, a0 + PIECE)
                    w = (a1 - a0) * 64
                    ub = upcp.tile([P, PIECE * 64], agdt, tag="ub", name=f"ub_{who}_{pi}")
                    uf = upcp.tile([P, PIECE * 64], tdt, tag="uf", name=f"uf_{who}_{pi}")
                    eng = nc.scalar if (ACTDMA and act_ok and pi % 2 == 1) else nc.sync
                    if gate is not None and pi < 2:
                        eng.dma_start(out=ub[0:1, 0:1], in_=gate)
                        eng.dma_start(out=uf[0:1, 0:1].bitcast(agdt)[:, 0:1], in_=gate)
                    eng.dma_start(
                        out=ub[:, :w].rearrange("p (a f) -> p a f", f=64),
                        in_=th[:].rearrange("(p a) f -> p a f", p=P)[:, a0:a1, :],
                    )
                    if act_ok and pi % 2 == 1:
                        nc.scalar.copy(out=uf[:, :w], in_=ub[:, :w])
                    else:
                        nc.vector.tensor_copy(out=uf[:, :w], in_=ub[:, :w])
                    eng.dma_start(
                        out=tf[:].rearrange("(p a) f -> p a f", p=P)[:, a0:a1, :],
                        in_=uf[:, :w].rearrange("p (a f) -> p a f", f=64),
                    )

            # ------------- conv2 gathers + PE reduce ------------------------
            _gctr = [0]

            def conv2_half(sh, regs2):
                tab = t0 if sh == 0 else t1
                icol = _icol_of[sh]
                for gi, (half, blks) in enumerate(groups):
                    if half != sh:
                        continue
                    nb = len(blks)
                    nidx = nb * P
                    S = nidx // 16
                    stg = gstagep.tile(
                        [P, GBLK * 64], tdt, tag="stg", name=f"stg_{sh}_{gi}"
                    )
                    nc.gpsimd.dma_gather(
                        stg[:, : nb * 64].rearrange("p (b d) -> p b d", d=64),
                        tab[:],
                        idx_sb[:, icol : icol + S],
                        nidx,
                        nidx,
                        64,
                        queue_num=_gctr[0] % GQ,
                    )
                    _gctr[0] += 1
                    icol += S
                    for (i, bank, pcol, r, fstart, fstop) in group_segs[gi]:
                        if PE_REDUCE:
                            nc.tensor.matmul(
                                regs2[bank][:, pcol : pcol + r * 64],
                                lhsT=ident_r[:],
                                rhs=stg[:, i * 64 : (i + r) * 64],
                                start=fstart,
                                stop=fstop,
                            )
                        else:
                            k, ch = blks[i]
                            a0 = ch * 64
                            if fstart:
                                nc.scalar.copy(
                                    out=acc2[:, a0 : a0 + r * 64],
                                    in_=stg[:, i * 64 : (i + r) * 64],
                                )
                            else:
                                nc.vector.tensor_add(
                                    acc2[:, a0 : a0 + r * 64],
                                    acc2[:, a0 : a0 + r * 64],
                                    stg[:, i * 64 : (i + r) * 64],
                                )

            # precompute idx column offsets per source half
            _icol_of = {0: 0, 1: 0}
            icol = 0
            for gi, (half, blks) in enumerate(groups):
                if half == 1 and _icol_of[1] == 0:
                    _icol_of[1] = icol
                icol += len(blks) * P // 16

            # =================== emission order =============================
            conv1_half(0)
            conv1_half(1)
            if AGBF:
                upconvert(t0h, t0, NCORES * H0, "t0", act_ok=False, gate=ag1_in[0:1, 0:1])

            if PE_REDUCE:
                regs2 = [
                    psump.tile([P, 512], f32, tag="ps", bufs=8, name=f"c2r_{b}")
                    for b in range(-(-nch // 8))
                ]
                acc2 = None
            else:
                regs2 = None
                acc2 = bigp.tile([P, nch * 64], f32, name="acc2", tag="acc2")

            conv2_half(0, regs2)
            if AGBF:
                upconvert(t1h, t1, NCORES * H1, "t1", act_ok=True)
            conv2_half(1, regs2)

            # h2 = tanh(acc2*dinv_nm + b2) node-major
            h2 = bigp.tile([P, nch * 64], f32, name="h2", tag="h2")
            for ch in range(nch):
                if PE_REDUCE:
                    src_ap = regs2[ch // 8][:, (ch % 8) * 64 : (ch % 8) * 64 + 64]
                else:
                    src_ap = acc2[:, ch * 64 : (ch + 1) * 64]
                nc.vector.scalar_tensor_tensor(
                    out=h2[:, ch * 64 : (ch + 1) * 64],
                    in0=src_ap,
                    scalar=dinv_nm[:, ch : ch + 1],
                    in1=b2e_sb[:],
                    op0=mybir.AluOpType.mult,
                    op1=mybir.AluOpType.add,
                )
            nc.scalar.activation(h2[:], h2[:], mybir.ActivationFunctionType.Tanh)

            # ------------- FC head -----------------------------------------
            h2fm = bigp.tile([64, shard], bf16, name="h2fm", tag="fm")
            for ch in range(nch):
                ptr = psump.tile([P, 512], f32, tag="ps", bufs=8, name=f"pst_{ch}")
                nc.tensor.transpose(
                    out=ptr[:64, :P],
                    in_=h2[:, ch * 64 : (ch + 1) * 64],
                    identity=ident[:],
                )
                nc.scalar.copy(out=h2fm[:, ch * P : (ch + 1) * P], in_=ptr[:64, :P])

            h3 = bigp.tile([32, shard], bf16, name="h3", tag="h3")
            for m0 in range(0, shard, 512):
                m1 = min(shard, m0 + 512)
                pf = psump.tile([P, 512], f32, tag="ps", bufs=8, name=f"psf_{m0}")
                nc.tensor.matmul(
                    pf[:32, : m1 - m0], lhsT=fw1_sb[:], rhs=h2fm[:, m0:m1],
                    start=True, stop=True,
                )
                nc.scalar.activation(
                    h3[:, m0:m1],
                    pf[:32, : m1 - m0],
                    mybir.ActivationFunctionType.Tanh,
                    bias=fb1_sb[:, :1],
                )
            ysb = smallp.tile([1, shard], f32, tag="ysb", bufs=1, name="ysb")
            for m0 in range(0, shard, 512):
                m1 = min(shard, m0 + 512)
                pg = psump.tile([P, 512], f32, tag="ps", bufs=8, name=f"psg_{m0}")
                nc.tensor.matmul(
                    pg[:1, : m1 - m0], lhsT=fw2_sb[:], rhs=h3[:, m0:m1],
                    start=True, stop=True,
                )
                nc.scalar.activation(
                    ysb[:, m0:m1],
                    pg[:1, : m1 - m0],
                    mybir.ActivationFunctionType.Copy,
                    bias=fb2,
                )
            nc.sync.dma_start(out=y_out.ap(), in_=ysb[:])

    nc.compile()
    return nc


# ---------------------------------------------------------------------------
# Entry point
# ---------------------------------------------------------------------------

def _in_maps(st, per_core, weights):
    w1dt = (
        ml_dtypes.float8_e4m3fn
        if os.environ.get("GCN_X8", "0") == "1"
        else ml_dtypes.bfloat16
    )
    w1 = np.asarray(weights["conv_w1"], np.float32).astype(w1dt)
    w2 = np.asarray(weights["conv_w2"], np.float32).astype(ml_dtypes.bfloat16)
    fw1 = np.asarray(weights["fc_w1"], np.float32).astype(ml_dtypes.bfloat16)
    fw2 = np.asarray(weights["fc_w2"], np.float32).astype(ml_dtypes.bfloat16)
    b1 = np.asarray(weights["conv_b1"], np.float32).reshape(64, 1)
    b2e = np.tile(np.asarray(weights["conv_b2"], np.float32)[None, :], (P, 1))
    fb1 = np.asarray(weights["fc_b1"], np.float32).reshape(32, 1)
    maps = []
    for c in range(NCORES):
        pc = per_core[c]
        maps.append(
            {
                "x_exp": pc["x_exp"],
                "idx2": pc["idx"],
                "dinv_fm": pc["dinv_fm"],
                "dinv_nm": pc["dinv_nm"],
                "w1": np.ascontiguousarray(w1),
                "w2": np.ascontiguousarray(w2),
                "fw1": np.ascontiguousarray(fw1),
                "fw2": np.ascontiguousarray(fw2),
                "b1c": b1,
                "b2e": b2e,
                "fb1c": fb1,
            }
        )
    return maps


def kernel(**inputs):
    x = np.asarray(inputs["x"], np.float32)
    edge_index = np.asarray(inputs["edge_index"])
    weights = {
        k: np.asarray(inputs[k], np.float32)
        for k in (
            "conv_w1",
            "conv_b1",
            "conv_w2",
            "conv_b2",
            "fc_w1",
            "fc_b1",
            "fc_w2",
            "fc_b2",
        )
    }
    st, per_core, dinv = _preprocess(x, edge_index)
    nc = _build(st, weights, n_passes=1)
    maps = _in_maps(st, per_core, weights)
    res = None
    for attempt in range(3):
        try:
            res = bass_utils.run_bass_kernel_spmd(
                nc, maps, core_ids=list(range(NCORES))
            )
            break
        except Exception as e:  # device wedge: retry
            if attempt == 2:
                raise
            print(f"[kernel] run attempt {attempt} failed ({e}); retrying")
    N, shard = st["N"], st["shard"]
    node_at = st["node_at"]
    y = np.empty((N, 1), np.float32)
    for c in range(NCORES):
        yc = res.results[c]["y"].reshape(shard)
        valid = node_at[c] >= 0
        y[node_at[c][valid], 0] = yc[valid]
    return y


# revision 28
# speedup vs baseline: 1.0310x; 1.0310x over previous
"""BrainGCN kernel for 8 Trainium2 NeuronCores (Bass/Tile).

Strategy (v2):
- Nodes partitioned across 8 cores (degree-sorted snake deal), shard=6272
  locals per core (49 chunks of 128). Each node is assigned to one of two
  "halves" (table windows) with a greedy per-destination balance pass so that
  every destination's in-edges split ~evenly between halves; this keeps the
  round-padded slot structure tight (~1.1x instead of 1.45x).
- conv1: host pre-expands x*dinv into per-edge-slot columns (bf16,
  feature-major, round-major within each half's chunk group). The device
  streams slabs and accumulates directly in PSUM: one [64,128] PSUM region
  per chunk, matmuls accumulate rounds (start on k==0), so no DVE adds.
  h1 = tanh(psum*dinv + b1) read out per bank.
- z2 = (h1*dinv) @ W2 per chunk (node-major), converted to bf16 and
  AllGathered per half as soon as that half's chunks finish, overlapping the
  collective with the other half's conv1 streaming. Gathered bf16 tables are
  upconverted on device to fp32 tables for dma_gather (256B rows).
- conv2: big dma_gather groups (GBLK blocks = GBLK*128 indices each) on 4
  SWDGE queues; reduction on the PE via fp32r identity-matmuls accumulating
  into per-chunk [128,64] PSUM regions (1 cycle/row), h2 read out per bank.
- FC head: per-chunk PE transposes, feature-major matmuls, fused tanh+bias.

kernel(**inputs) takes FULL inputs, preprocesses + shards on host, compiles
and runs the SPMD program on cores 0..7, and reassembles the full output.
"""

import os
import warnings

warnings.filterwarnings("ignore")

import numpy as np
import ml_dtypes

from concourse import bacc, bass, mybir, tile
from concourse.masks import make_identity
import concourse.bass_utils as bass_utils

P = 128
NCORES = 8
GQ = int(os.environ.get("GCN_GQ", "4"))  # SWDGE queues for conv2 gathers
GBLK = int(os.environ.get("GCN_GBLK", "8"))  # blocks per dma_gather
SLAB = int(os.environ.get("GCN_SLAB", "8192"))  # conv1 stream columns per DMA
PE_REDUCE = bool(int(os.environ.get("GCN_PE_REDUCE", "1")))
AG8 = bool(int(os.environ.get("GCN_AG8", "0")))  # fp8 AllGather payload
X8 = bool(int(os.environ.get("GCN_X8", "0")))  # fp8 conv1 stream
F32R = bool(int(os.environ.get("GCN_F32R", "1")))  # fp32r PE reduce pipeline
ACTDMA = bool(int(os.environ.get("GCN_ACTDMA", "1")))  # allow ACT-engine DMAs
AGBF = bool(int(os.environ.get("GCN_AGBF", "1")))  # bf16 AllGather payload + upconvert
C1PSUM = bool(int(os.environ.get("GCN_C1PSUM", "1")))  # conv1 PSUM-region accumulate
ZSCALE = bool(int(os.environ.get("GCN_ZSCALE", "1")))  # fused dinv scale on z2 copy


# ---------------------------------------------------------------------------
# Host preprocessing
# ---------------------------------------------------------------------------

def _preprocess(x, edge_index):
    """Partition nodes, build slot structures and per-core input arrays."""
    N = x.shape[0]
    E = edge_index.shape[1]
    src = np.asarray(edge_index[0], dtype=np.int64)
    dst = np.asarray(edge_index[1], dtype=np.int64)

    shard = -(-N // (NCORES * P)) * P  # 6272
    nch = shard // P  # 49
    h0ch = (nch + 1) // 2  # 25
    h1ch = nch - h0ch  # 24
    H0 = h0ch * P  # 3200 positions per core in half 0
    H1 = h1ch * P  # 3072

    deg = 1 + np.bincount(dst, minlength=N)  # includes self-loop
    dinv = (1.0 / np.sqrt(deg)).astype(np.float32)

    counts = np.array([N // NCORES + (c < N % NCORES) for c in range(NCORES)])
    # phase A: global degree sort (desc), snake deal to cores
    order = np.argsort(-deg, kind="stable")
    core_of = np.empty(N, np.int32)
    taken = np.zeros(NCORES, np.int64)
    ci = 0
    direction = 1
    for v in order:
        for _ in range(NCORES):
            if taken[ci] < counts[ci]:
                break
            ci = (ci + direction) % NCORES
        core_of[v] = ci
        taken[ci] += 1
        ci += direction
        if ci == NCORES:
            ci, direction = NCORES - 1, -1
        elif ci == -1:
            ci, direction = 0, 1

    # out-adjacency (dests per source), for the balance pass
    sorder = np.argsort(src, kind="stable")
    src_ss = src[sorder]
    dst_ss = dst[sorder]
    ostarts = np.searchsorted(src_ss, np.arange(N))
    oends = np.searchsorted(src_ss, np.arange(N) + 1)

    # greedy balanced half assignment: process nodes in degree-desc order;
    # put v in the half where its out-destinations currently have fewer
    # sources, respecting per-(core,half) capacity (one pad row reserved).
    d0 = np.zeros(N, np.int32)
    d1 = np.zeros(N, np.int32)
    n0 = np.zeros(NCORES, np.int64)
    n1 = np.zeros(NCORES, np.int64)
    cap0, cap1 = H0 - 1, H1 - 1
    half_of = np.empty(N, np.int8)
    for v in order:
        c = core_of[v]
        D = dst_ss[ostarts[v]:oends[v]]
        s0 = int(d0[D].sum()) + int(d0[v])  # self-loop dest is v itself
        s1 = int(d1[D].sum()) + int(d1[v])
        h = 0 if s0 <= s1 else 1
        if h == 0 and n0[c] >= cap0:
            h = 1
        elif h == 1 and n1[c] >= cap1:
            h = 0
        half_of[v] = h
        if h == 0:
            np.add.at(d0, D, 1)
            d0[v] += 1
            n0[c] += 1
        else:
            np.add.at(d1, D, 1)
            d1[v] += 1
            n1[c] += 1
    assert (n0 <= cap0).all() and (n1 <= cap1).all()

    # refinement sweeps: re-assign each node to the half that best balances
    # its destinations' source counts (capacity-respecting)
    for _ in range(3):
        moved = 0
        for v in order:
            c = core_of[v]
            D = dst_ss[ostarts[v]:oends[v]]
            h_cur = half_of[v]
            # counts with v removed
            if h_cur == 0:
                np.add.at(d0, D, -1)
                d0[v] -= 1
                n0[c] -= 1
            else:
                np.add.at(d1, D, -1)
                d1[v] -= 1
                n1[c] -= 1
            s0 = int(d0[D].sum()) + int(d0[v])
            s1 = int(d1[D].sum()) + int(d1[v])
            h = 0 if s0 <= s1 else 1
            if h == 0 and n0[c] >= cap0:
                h = 1
            elif h == 1 and n1[c] >= cap1:
                h = 0
            if h != h_cur:
                moved += 1
            half_of[v] = h
            if h == 0:
                np.add.at(d0, D, 1)
                d0[v] += 1
                n0[c] += 1
            else:
                np.add.at(d1, D, 1)
                d1[v] += 1
                n1[c] += 1
        if moved == 0:
            break

    # phase B: positions within each (core, half) by total degree desc
    pos_of = np.full(N, -1, np.int64)
    for c in range(NCORES):
        for h, base in ((0, 0), (1, H0)):
            mask = (core_of == c) & (half_of == h)
            mem = np.nonzero(mask)[0]
            key = np.lexsort((-(d0[mem] if h == 0 else d1[mem]), -deg[mem]))
            pos_of[mem[key]] = base + np.arange(len(mem))

    # global half-table rows
    grow_h = np.where(
        half_of == 0,
        core_of.astype(np.int64) * H0 + pos_of,
        core_of.astype(np.int64) * H1 + (pos_of - H0),
    )

    # per-node in-edge src lists
    eorder = np.argsort(dst, kind="stable")
    dst_s = dst[eorder]
    src_s = src[eorder]
    starts = np.searchsorted(dst_s, np.arange(N))
    ends = np.searchsorted(dst_s, np.arange(N) + 1)

    # round counts per chunk (global max over cores)
    # conv1: total degree; conv2: per-source-half degree of the dest
    Ktg = np.zeros(nch, np.int32)
    K0g = np.zeros(nch, np.int32)
    K1g = np.zeros(nch, np.int32)
    ch_of = (pos_of // P).astype(np.int32)
    np.maximum.at(Ktg, ch_of, deg.astype(np.int32))
    np.maximum.at(K0g, ch_of, d0)
    np.maximum.at(K1g, ch_of, d1)
    assert (K0g >= 1).all() and (K1g >= 1).all() and (Ktg >= 1).all()

    # conv1 block list: half-group-major, then round-major
    def round_major(Karr, chs):
        kmax = int(Karr[chs].max()) if len(chs) else 0
        blocks = []
        for k in range(kmax):
            for ch in chs:
                if Karr[ch] > k:
                    blocks.append((k, ch))
        return blocks

    blocks1_g0 = round_major(Ktg, list(range(h0ch)))
    blocks1_g1 = round_major(Ktg, list(range(h0ch, nch)))
    blocks1 = blocks1_g0 + blocks1_g1
    nb1_g0 = len(blocks1_g0)
    S1 = len(blocks1) * P

    # conv2 block lists per source half, round-major over all chunks
    blocks2 = {
        0: round_major(K0g, list(range(nch))),
        1: round_major(K1g, list(range(nch))),
    }

    # conv2 gather groups (<= GBLK blocks each, within one source half)
    groups = []
    for half in (0, 1):
        blks = blocks2[half]
        for i in range(0, len(blks), GBLK):
            groups.append((half, blks[i : i + GBLK]))

    tot2 = (len(blocks2[0]) + len(blocks2[1])) * P
    per_core_work = (E + N) / NCORES
    print(
        f"[pre] shard={shard} conv1 slots={S1} ({S1/per_core_work:.3f}x) "
        f"conv2 slots={tot2} ({tot2/per_core_work:.3f}x) groups={len(groups)}"
    )

    node_at = np.full((NCORES, shard), -1, np.int64)
    node_at[core_of, pos_of] = np.arange(N)

    # PE reduce segments for conv2: runs of consecutive ch at the same k
    # within one PSUM bank. PSUM accumulation groups are bank-granular
    # (ZERO_REGION=2KB): exactly one start (first segment touching the bank,
    # which zeroes the whole bank) and one stop (last segment) per bank.
    group_segs = []
    for half, blks in groups:
        segs = []
        i = 0
        while i < len(blks):
            k, ch = blks[i]
            r = 1
            while (
                i + r < len(blks)
                and blks[i + r] == (k, ch + r)
                and (ch + r) // 8 == ch // 8
            ):
                r += 1
            segs.append([i, ch // 8, (ch % 8) * 64, r, False, False])
            i += r
        group_segs.append(segs)
    seen_bank = set()
    last_seg_of_bank = {}
    for gi, segs in enumerate(group_segs):
        for si_, seg in enumerate(segs):
            b = seg[1]
            if b not in seen_bank:
                seen_bank.add(b)
                seg[4] = True
            last_seg_of_bank[b] = (gi, si_)
    for b, (gi, si_) in last_seg_of_bank.items():
        group_segs[gi][si_][5] = True

    # conv1 PE segments: per 512-col matmul window of each slab we need runs
    # of consecutive ch within one conv1 PSUM bank (4 regions of [64,128]).
    # Built on the fly in _build from blocks1.

    xdt_np = (
        ml_dtypes.float8_e4m3fn
        if os.environ.get("GCN_X8", "0") == "1"
        else ml_dtypes.bfloat16
    )
    xs = x.astype(np.float32) * dinv[:, None]
    xsT = np.ascontiguousarray(xs.T).astype(xdt_np)  # [128, N]

    # per-core slot source arrays
    b1_of = {}
    for i, (k, ch) in enumerate(blocks1):
        b1_of[(k, ch)] = i
    b2_of = {0: {}, 1: {}}
    for h in (0, 1):
        for i, (k, ch) in enumerate(blocks2[h]):
            b2_of[h][(k, ch)] = i

    per_core = []
    for c in range(NCORES):
        src1 = np.full((len(blocks1), P), -1, np.int64)
        idx2 = {
            0: np.full((len(blocks2[0]), P), -1, np.int64),
            1: np.full((len(blocks2[1]), P), -1, np.int64),
        }
        zero_row = {0: c * H0 + H0 - 1, 1: c * H1 + H1 - 1}
        for pos in range(shard):
            v = node_at[c, pos]
            if v < 0:
                continue
            ch, p = pos // P, pos % P
            e0 = src_s[starts[v] : ends[v]]
            halves = half_of[e0]
            l0 = e0[halves == 0]
            l1 = e0[halves == 1]
            if half_of[v] == 0:
                l0 = np.concatenate(([v], l0))
            else:
                l1 = np.concatenate(([v], l1))
            ltot = np.concatenate((l0, l1))
            for k in range(len(ltot)):
                src1[b1_of[(k, ch)], p] = ltot[k]
            for h, lh in ((0, l0), (1, l1)):
                for k in range(len(lh)):
                    idx2[h][b2_of[h][(k, ch)], p] = grow_h[lh[k]]

        flat1 = src1.reshape(-1)
        x_exp = np.zeros((P, S1), dtype=xdt_np)
        valid = flat1 >= 0
        x_exp[:, valid] = xsT[:, flat1[valid]]

        slabs = []
        for gi, (half, blks) in enumerate(groups):
            idxs = np.empty((len(blks), P), np.int64)
            for j, (k, ch) in enumerate(blks):
                row = idx2[half][b2_of[half][(k, ch)]]
                idxs[j] = np.where(row >= 0, row, zero_row[half])
            flat = idxs.reshape(-1)
            assert flat.max() < 32768, flat.max()
            S = len(flat) // 16
            wrapped = flat.reshape(S, 16).T.astype(np.int16)  # [16, S]
            slabs.append(wrapped)
        idx_cat = np.concatenate(slabs, axis=1)
        idx_rep = np.tile(idx_cat, (8, 1))  # [128, sum S]

        dinv_loc = np.zeros(shard, np.float32)
        valid_pos = node_at[c] >= 0
        dinv_loc[valid_pos] = dinv[node_at[c][valid_pos]]
        dinv_fm = np.tile(dinv_loc[None, :], (64, 1)).astype(ml_dtypes.bfloat16)
        dinv_nm = dinv_loc.reshape(nch, P).T.astype(np.float32).copy()  # [128,nch]

        per_core.append(
            dict(x_exp=x_exp, idx=idx_rep, dinv_fm=dinv_fm, dinv_nm=dinv_nm)
        )

    struct = dict(
        N=N,
        shard=shard,
        nch=nch,
        h0ch=h0ch,
        h1ch=h1ch,
        H0=H0,
        H1=H1,
        blocks1=blocks1,
        nb1_g0=nb1_g0,
        groups=groups,
        group_segs=group_segs,
        blocks2=blocks2,
        S1=S1,
        node_at=node_at,
        idx_cols=per_core[0]["idx"].shape[1],
        K1g=K1g,
    )
    return struct, per_core, dinv


# ---------------------------------------------------------------------------
# Program builder
# ---------------------------------------------------------------------------

def _build(st, weights, n_passes=1):
    """Build the SPMD Bass program."""
    shard, nch = st["shard"], st["nch"]
    S1 = st["S1"]
    blocks1 = st["blocks1"]
    nb1_g0 = st["nb1_g0"]
    groups = st["groups"]
    group_segs = st["group_segs"]
    H0, H1 = st["H0"], st["H1"]
    h0ch, h1ch = st["h0ch"], st["h1ch"]
    bf16 = mybir.dt.bfloat16
    f32 = mybir.dt.float32
    f32r = mybir.dt.float32r
    tdt = f32r if (F32R and PE_REDUCE) else f32
    agdt = (mybir.dt.float8e4 if AG8 else bf16) if AGBF else tdt
    xdt = mybir.dt.float8e4 if X8 else bf16

    fb2 = float(np.asarray(weights["fc_b2"]).reshape(-1)[0])

    nc = bacc.Bacc(
        "TRN2",
        target_bir_lowering=False,
        debug=False,
        enable_asserts=False,
        num_devices=NCORES,
        num_swdge_queues=GQ,
    )

    x_exp_in = nc.dram_tensor("x_exp", [P, S1], xdt, kind="ExternalInput")
    idx_in = nc.dram_tensor(
        "idx2", [P, st["idx_cols"]], mybir.dt.int16, kind="ExternalInput"
    )
    dinv_fm_in = nc.dram_tensor("dinv_fm", [64, shard], bf16, kind="ExternalInput")
    dinv_nm_in = nc.dram_tensor("dinv_nm", [P, nch], f32, kind="ExternalInput")
    w1_in = nc.dram_tensor("w1", [P, 64], xdt, kind="ExternalInput")
    w2_in = nc.dram_tensor("w2", [64, 64], bf16, kind="ExternalInput")
    fw1_in = nc.dram_tensor("fw1", [64, 32], bf16, kind="ExternalInput")
    fw2_in = nc.dram_tensor("fw2", [32, 1], bf16, kind="ExternalInput")
    b1_in = nc.dram_tensor("b1c", [64, 1], f32, kind="ExternalInput")
    b2e_in = nc.dram_tensor("b2e", [P, 64], f32, kind="ExternalInput")
    fb1_in = nc.dram_tensor("fb1c", [32, 1], f32, kind="ExternalInput")
    y_out = nc.dram_tensor("y", [1, shard], f32, kind="ExternalOutput")

    with tile.TileContext(nc) as tc:
        with (
            tc.tile_pool(name="const", bufs=1) as constp,
            tc.tile_pool(name="big", bufs=1) as bigp,
            tc.tile_pool(name="xslab", bufs=3 if C1PSUM else 2) as xslabp,
            tc.tile_pool(name="gstage", bufs=3) as gstagep,
            tc.tile_pool(name="upc", bufs=2) as upcp,
            tc.tile_pool(name="psum", bufs=8, space="PSUM") as psump,
            tc.tile_pool(name="small", bufs=2) as smallp,
            tc.tile_pool(name="dram", bufs=1, space="DRAM") as dramp,
        ):
            # constants
            w1_sb = constp.tile([P, 64], xdt, name="w1_sb")
            nc.sync.dma_start(out=w1_sb[:], in_=w1_in.ap())
            w2_sb = constp.tile([64, 64], bf16, name="w2_sb")
            nc.sync.dma_start(out=w2_sb[:], in_=w2_in.ap())
            fw1_sb = constp.tile([64, 32], bf16, name="fw1_sb")
            nc.sync.dma_start(out=fw1_sb[:], in_=fw1_in.ap())
            fw2_sb = constp.tile([32, 1], bf16, name="fw2_sb")
            nc.sync.dma_start(out=fw2_sb[:], in_=fw2_in.ap())
            b1_sb = constp.tile([64, 1], f32, name="b1_sb")
            nc.sync.dma_start(out=b1_sb[:], in_=b1_in.ap())
            b2e_sb = constp.tile([P, 64], f32, name="b2e_sb")
            nc.sync.dma_start(out=b2e_sb[:], in_=b2e_in.ap())
            fb1_sb = constp.tile([32, 1], f32, name="fb1_sb")
            nc.sync.dma_start(out=fb1_sb[:], in_=fb1_in.ap())
            dinv_fm = constp.tile([64, shard], bf16, name="dinv_fm_sb")
            nc.sync.dma_start(out=dinv_fm[:], in_=dinv_fm_in.ap())
            dinv_nm = constp.tile([P, nch], f32, name="dinv_nm_sb")
            nc.sync.dma_start(out=dinv_nm[:], in_=dinv_nm_in.ap())
            ident = constp.tile([P, P], f32, name="ident")
            make_identity(nc, ident[:])
            idx_sb = constp.tile([P, st["idx_cols"]], mybir.dt.int16, name="idx_sb")
            if F32R and PE_REDUCE:
                ident_r = constp.tile([P, P], f32r, name="ident_r")
                nc.vector.tensor_copy(out=ident_r[:], in_=ident[:])
            else:
                ident_r = ident

            # DRAM tables
            t0h = dramp.tile([NCORES * H0, 64], agdt, name="t0h", addr_space="Shared")
            t1h = dramp.tile([NCORES * H1, 64], agdt, name="t1h", addr_space="Shared")
            if AGBF:
                t0 = dramp.tile([NCORES * H0, 64], tdt, name="t0")
                t1 = dramp.tile([NCORES * H1, 64], tdt, name="t1")
            else:
                t0, t1 = t0h, t1h
            ag0_in = dramp.tile([H0, 64], agdt, name="ag0i")
            ag1_in = dramp.tile([H1, 64], agdt, name="ag1i")

            h1s = bigp.tile([64, shard], bf16, name="h1s", tag="fm")
            acc1 = None
            if not C1PSUM:
                acc1 = bigp.tile([64, shard], f32, name="acc1", tag="acc1")

            # per-chunk conv1 round counts from blocks1
            _KT = {}
            for k, ch in blocks1:
                _KT[ch] = max(_KT.get(ch, 0), k + 1)

            # ------------- conv1 (per half group) + z2 + AllGather ----------
            def conv1_half(hg):
                b_lo = 0 if hg == 0 else nb1_g0
                b_hi = nb1_g0 if hg == 0 else len(blocks1)
                ch_lo = 0 if hg == 0 else h0ch
                nch_g = h0ch if hg == 0 else h1ch
                # PSUM regions: 4 chunks of [64,128] per bank
                nbank = -(-nch_g // 4)
                regs = []
                if C1PSUM:
                    for b in range(nbank):
                        rg = psump.tile(
                            [P, 512], f32, tag="ps", bufs=8, name=f"c1r_{hg}_{b}"
                        )
                        regs.append(rg)

                c_lo, c_hi = b_lo * P, b_hi * P
                n_slabs = -(-(c_hi - c_lo) // SLAB)
                # pre-enumerate matmul segments: (slab, i, r, bank, col)
                seg_list = []
                for si in range(n_slabs):
                    s0 = c_lo + si * SLAB
                    s1_ = min(c_hi, s0 + SLAB)
                    i = s0 // P
                    bend = s1_ // P
                    while i < bend:
                        k, ch = blocks1[i]
                        lch = ch - ch_lo
                        r = 1
                        while (
                            i + r < bend
                            and blocks1[i + r] == (k, ch + r)
                            and (lch + r) // 4 == lch // 4
                        ):
                            r += 1
                        seg_list.append([si, i, r, lch // 4, (lch % 4) * P])
                        i += r
                seen_b = set()
                last_of_b = {}
                flags = []
                for j, (si, i, r, b, col) in enumerate(seg_list):
                    st_f = b not in seen_b
                    seen_b.add(b)
                    last_of_b[b] = j
                    flags.append([st_f, False])
                for b, j in last_of_b.items():
                    flags[j][1] = True

                jseg = 0
                for si in range(n_slabs):
                    s0 = c_lo + si * SLAB
                    s1_ = min(c_hi, s0 + SLAB)
                    xsl = xslabp.tile([P, SLAB], xdt, tag="xsl", name=f"xsl_{hg}_{si}")
                    eng = nc.sync if (si % 2 == 0 or not ACTDMA) else nc.scalar
                    eng.dma_start(out=xsl[:, : s1_ - s0], in_=x_exp_in.ap()[:, s0:s1_])
                    if C1PSUM:
                        while jseg < len(seg_list) and seg_list[jseg][0] == si:
                            _, i, r, b, col = seg_list[jseg]
                            st_f, sp_f = flags[jseg]
                            nc.tensor.matmul(
                                regs[b][:64, col : col + r * P],
                                lhsT=w1_sb[:],
                                rhs=xsl[:, (i * P - s0) : (i * P - s0) + r * P],
                                start=st_f,
                                stop=sp_f,
                            )
                            jseg += 1
                    else:
                        for m0 in range(s0, s1_, 512):
                            m1 = min(s1_, m0 + 512)
                            pt = psump.tile(
                                [P, 512], f32, tag="ps", bufs=8, name=f"ps1_{hg}_{m0}"
                            )
                            nc.tensor.matmul(
                                pt[:64, : m1 - m0],
                                lhsT=w1_sb[:],
                                rhs=xsl[:, m0 - s0 : m1 - s0],
                                start=True,
                                stop=True,
                            )
                            b0, bend2 = m0 // P, m1 // P
                            i = b0
                            while i < bend2:
                                k, ch = blocks1[i]
                                r = 1
                                while i + r < bend2 and blocks1[i + r] == (k, ch + r):
                                    r += 1
                                if k == 0:
                                    nc.scalar.copy(
                                        out=acc1[:, ch * P : ch * P + r * P],
                                        in_=pt[:64, (i - b0) * P : (i - b0 + r) * P],
                                    )
                                else:
                                    nc.vector.tensor_add(
                                        acc1[:, ch * P : ch * P + r * P],
                                        acc1[:, ch * P : ch * P + r * P],
                                        pt[:64, (i - b0) * P : (i - b0 + r) * P],
                                    )
                                i += r

                # h1 = tanh(acc*dinv + b1), per bank readout
                cols0 = ch_lo * P
                for b in range(nbank):
                    w = min(512, (nch_g - b * 4) * P)
                    a0 = cols0 + b * 512
                    nc.vector.tensor_mul(
                        h1s[:, a0 : a0 + w],
                        regs[b][:64, :w] if C1PSUM else acc1[:, a0 : a0 + w],
                        dinv_fm[:, a0 : a0 + w],
                    )
                gw = nch_g * P
                nc.scalar.activation(
                    h1s[:, cols0 : cols0 + gw],
                    h1s[:, cols0 : cols0 + gw],
                    mybir.ActivationFunctionType.Tanh,
                    bias=b1_sb[:, :1],
                )

                # z2 (node-major bf16) for this half's chunks
                z2st = smallp.tile(
                    [P, nch_g * 64], agdt, tag="z2st", name=f"z2st_{hg}"
                )
                for j in range(nch_g):
                    ch = ch_lo + j
                    pz = psump.tile(
                        [P, 512], f32, tag="ps", bufs=8, name=f"pz_{hg}_{j}"
                    )
                    nc.tensor.matmul(
                        pz[:, :64],
                        lhsT=h1s[:, ch * P : (ch + 1) * P],
                        rhs=w2_sb[:],
                        start=True,
                        stop=True,
                    )
                    # z2 = (h1 @ W2) * dinv (per-node row scale, fused here)
                    if not ZSCALE:
                        nc.vector.tensor_scalar_mul(
                            z2st[:, j * 64 : (j + 1) * 64],
                            pz[:, :64],
                            dinv_nm[:, ch : ch + 1],
                        )
                    elif j % 2 == 0:
                        nc.scalar.activation(
                            z2st[:, j * 64 : (j + 1) * 64],
                            pz[:, :64],
                            mybir.ActivationFunctionType.Copy,
                            scale=dinv_nm[:, ch : ch + 1],
                        )
                    else:
                        nc.vector.tensor_scalar_mul(
                            z2st[:, j * 64 : (j + 1) * 64],
                            pz[:, :64],
                            dinv_nm[:, ch : ch + 1],
                        )

                ag_in = ag0_in if hg == 0 else ag1_in
                th = t0h if hg == 0 else t1h
                (nc.scalar if ACTDMA else nc.sync).dma_start(
                    out=ag_in[:].rearrange("(c p) f -> p c f", p=P),
                    in_=z2st[:].rearrange("p (c f) -> p c f", f=64),
                )
                nc.gpsimd.collective_compute(
                    "AllGather",
                    mybir.AluOpType.bypass,
                    replica_groups=[list(range(NCORES))],
                    ins=[ag_in.opt()],
                    outs=[th.opt()],
                )

            # upconvert one bf16 half-table to fp32, in pieces.
            # partition-major view: partition p holds table rows
            # [p*a_tot, (p+1)*a_tot) so each DMA is 128 contiguous runs.
            def upconvert(th, tf, rows, who, act_ok, gate=None):
                # act_ok=False keeps the ACT queue clear (so a later ag DMA
                # is not stuck behind pieces that wait on this AllGather).
                # gate: tiny DRAM AP whose write must precede this phase —
                # read one element into the staging tiles so the scheduler
                # cannot hoist these pieces ahead of the gate's producer.
                a_tot = rows // P
                PIECE = -(-a_tot // 8)
                for pi, a0 in enumerate(range(0, a_tot, PIECE)):
                    a1 = min(a_tot, a0 + PIECE)
                    w = (a1 - a0) * 64
                    ub = upcp.tile([P, PIECE * 64], agdt, tag="ub", name=f"ub_{who}_{pi}")
                    uf = upcp.tile([P, PIECE * 64], tdt, tag="uf", name=f"uf_{who}_{pi}")
                    eng = nc.scalar if (ACTDMA and act_ok and pi % 2 == 1) else nc.sync
                    if gate is not None and pi < 2:
                        eng.dma_start(out=ub[0:1, 0:1], in_=gate)
                        eng.dma_start(out=uf[0:1, 0:1].bitcast(agdt)[:, 0:1], in_=gate)
                    eng.dma_start(
                        out=ub[:, :w].rearrange("p (a f) -> p a f", f=64),
                        in_=th[:].rearrange("(p a) f -> p a f", p=P)[:, a0:a1, :],
                    )
                    if act_ok and pi % 2 == 1:
                        nc.scalar.copy(out=uf[:, :w], in_=ub[:, :w])
                    else:
                        nc.vector.tensor_copy(out=uf[:, :w], in_=ub[:, :w])
                    eng.dma_start(
                        out=tf[:].rearrange("(p a) f -> p a f", p=P)[:, a0:a1, :],
                        in_=uf[:, :w].rearrange("p (a f) -> p a f", f=64),
                    )

            # ------------- conv2 gathers + PE reduce ------------------------
            _gctr = [0]

            def conv2_half(sh, regs2):
                tab = t0 if sh == 0 else t1
                icol = _icol_of[sh]
                for gi, (half, blks) in enumerate(groups):
                    if half != sh:
                        continue
                    nb = len(blks)
                    nidx = nb * P
                    S = nidx // 16
                    stg = gstagep.tile(
                        [P, GBLK * 64], tdt, tag="stg", name=f"stg_{sh}_{gi}"
                    )
                    nc.gpsimd.dma_gather(
                        stg[:, : nb * 64].rearrange("p (b d) -> p b d", d=64),
                        tab[:],
                        idx_sb[:, icol : icol + S],
                        nidx,
                        nidx,
                        64,
                        queue_num=_gctr[0] % GQ,
                    )
                    _gctr[0] += 1
                    icol += S
                    for (i, bank, pcol, r, fstart, fstop) in group_segs[gi]:
                        if PE_REDUCE:
                            nc.tensor.matmul(
                                regs2[bank][:, pcol : pcol + r * 64],
                                lhsT=ident_r[:],
                                rhs=stg[:, i * 64 : (i + r) * 64],
                                start=fstart,
                                stop=fstop,
                            )
                        else:
                            k, ch = blks[i]
                            a0 = ch * 64
                            if fstart:
                                nc.scalar.copy(
                                    out=acc2[:, a0 : a0 + r * 64],
                                    in_=stg[:, i * 64 : (i + r) * 64],
                                )
                            else:
                                nc.vector.tensor_add(
                                    acc2[:, a0 : a0 + r * 64],
                                    acc2[:, a0 : a0 + r * 64],
                                    stg[:, i * 64 : (i + r) * 64],
                                )

            # precompute idx column offsets per source half
            _icol_of = {0: 0, 1: 0}
            icol = 0
            for gi, (half, blks) in enumerate(groups):
                if half == 1 and _icol_of[1] == 0:
                    _icol_of[1] = icol
                icol += len(blks) * P // 16

            # =================== emission order =============================
            conv1_half(0)
            # idx table is not needed until the conv2 gathers; load it in the
            # SP idle window so it does not delay the first conv1 slab
            nc.sync.dma_start(out=idx_sb[:], in_=idx_in.ap())
            conv1_half(1)
            if AGBF:
                upconvert(t0h, t0, NCORES * H0, "t0", act_ok=False, gate=ag1_in[0:1, 0:1])

            if PE_REDUCE:
                regs2 = [
                    psump.tile([P, 512], f32, tag="ps", bufs=8, name=f"c2r_{b}")
                    for b in range(-(-nch // 8))
                ]
                acc2 = None
            else:
                regs2 = None
                acc2 = bigp.tile([P, nch * 64], f32, name="acc2", tag="acc2")

            conv2_half(0, regs2)
            if AGBF:
                upconvert(t1h, t1, NCORES * H1, "t1", act_ok=True)
            conv2_half(1, regs2)

            # h2 = tanh(acc2*dinv_nm + b2) node-major
            h2 = bigp.tile([P, nch * 64], f32, name="h2", tag="h2")
            for ch in range(nch):
                if PE_REDUCE:
                    src_ap = regs2[ch // 8][:, (ch % 8) * 64 : (ch % 8) * 64 + 64]
                else:
                    src_ap = acc2[:, ch * 64 : (ch + 1) * 64]
                nc.vector.scalar_tensor_tensor(
                    out=h2[:, ch * 64 : (ch + 1) * 64],
                    in0=src_ap,
                    scalar=dinv_nm[:, ch : ch + 1],
                    in1=b2e_sb[:],
                    op0=mybir.AluOpType.mult,
                    op1=mybir.AluOpType.add,
                )
            nc.scalar.activation(h2[:], h2[:], mybir.ActivationFunctionType.Tanh)

            # ------------- FC head -----------------------------------------
            h2fm = bigp.tile([64, shard], bf16, name="h2fm", tag="fm")
            for ch in range(nch):
                ptr = psump.tile([P, 512], f32, tag="ps", bufs=8, name=f"pst_{ch}")
                nc.tensor.transpose(
                    out=ptr[:64, :P],
                    in_=h2[:, ch * 64 : (ch + 1) * 64],
                    identity=ident[:],
                )
                if ch % 2 == 0:
                    nc.scalar.copy(out=h2fm[:, ch * P : (ch + 1) * P], in_=ptr[:64, :P])
                else:
                    nc.vector.tensor_copy(out=h2fm[:, ch * P : (ch + 1) * P], in_=ptr[:64, :P])

            h3 = bigp.tile([32, shard], bf16, name="h3", tag="h3")
            for m0 in range(0, shard, 512):
                m1 = min(shard, m0 + 512)
                pf = psump.tile([P, 512], f32, tag="ps", bufs=8, name=f"psf_{m0}")
                nc.tensor.matmul(
                    pf[:32, : m1 - m0], lhsT=fw1_sb[:], rhs=h2fm[:, m0:m1],
                    start=True, stop=True,
                )
                nc.scalar.activation(
                    h3[:, m0:m1],
                    pf[:32, : m1 - m0],
                    mybir.ActivationFunctionType.Tanh,
                    bias=fb1_sb[:, :1],
                )
            ysb = smallp.tile([1, shard], f32, tag="ysb", bufs=1, name="ysb")
            for m0 in range(0, shard, 512):
                m1 = min(shard, m0 + 512)
                pg = psump.tile([P, 512], f32, tag="ps", bufs=8, name=f"psg_{m0}")
                nc.tensor.matmul(
                    pg[:1, : m1 - m0], lhsT=fw2_sb[:], rhs=h3[:, m0:m1],
                    start=True, stop=True,
                )
                if (m0 // 512) % 2 == 0:
                    nc.scalar.activation(
                        ysb[:, m0:m1],
                        pg[:1, : m1 - m0],
                        mybir.ActivationFunctionType.Copy,
                        bias=fb2,
                    )
                else:
                    nc.vector.tensor_scalar_add(ysb[:, m0:m1], pg[:1, : m1 - m0], fb2)
            nc.sync.dma_start(out=y_out.ap(), in_=ysb[:])

    nc.compile()
    return nc


# ---------------------------------------------------------------------------
# Entry point
# ---------------------------------------------------------------------------

def _in_maps(st, per_core, weights):
    w1dt = (
        ml_dtypes.float8_e4m3fn
        if os.environ.get("GCN_X8", "0") == "1"
        else ml_dtypes.bfloat16
    )
    w1 = np.asarray(weights["conv_w1"], np.float32).astype(w1dt)
    w2 = np.asarray(weights["conv_w2"], np.float32).astype(ml_dtypes.bfloat16)
    fw1 = np.asarray(weights["fc_w1"], np.float32).astype(ml_dtypes.bfloat16)
    fw2 = np.asarray(weights["fc_w2"], np.float32).astype(ml_dtypes.bfloat16)
    b1 = np.asarray(weights["conv_b1"], np.float32).reshape(64, 1)
    b2e = np.tile(np.asarray(weights["conv_b2"], np.float32)[None, :], (P, 1))
    fb1 = np.asarray(weights["fc_b1"], np.float32).reshape(32, 1)
    maps = []
    for c in range(NCORES):
        pc = per_core[c]
        maps.append(
            {
                "x_exp": pc["x_exp"],
                "idx2": pc["idx"],
                "dinv_fm": pc["dinv_fm"],
                "dinv_nm": pc["dinv_nm"],
                "w1": np.ascontiguousarray(w1),
                "w2": np.ascontiguousarray(w2),
                "fw1": np.ascontiguousarray(fw1),
                "fw2": np.ascontiguousarray(fw2),
                "b1c": b1,
                "b2e": b2e,
                "fb1c": fb1,
            }
        )
    return maps


def kernel(**inputs):
    x = np.asarray(inputs["x"], np.float32)
    edge_index = np.asarray(inputs["edge_index"])
    weights = {
        k: np.asarray(inputs[k], np.float32)
        for k in (
            "conv_w1",
            "conv_b1",
            "conv_w2",
            "conv_b2",
            "fc_w1",
            "fc_b1",
            "fc_w2",
            "fc_b2",
        )
    }
    st, per_core, dinv = _preprocess(x, edge_index)
    nc = _build(st, weights, n_passes=1)
    maps = _in_maps(st, per_core, weights)
    res = None
    for attempt in range(3):
        try:
            res = bass_utils.run_bass_kernel_spmd(
                nc, maps, core_ids=list(range(NCORES))
            )
            break
        except Exception as e:  # device wedge: retry
            if attempt == 2:
                raise
            print(f"[kernel] run attempt {attempt} failed ({e}); retrying")
    N, shard = st["N"], st["shard"]
    node_at = st["node_at"]
    y = np.empty((N, 1), np.float32)
    for c in range(NCORES):
        yc = res.results[c]["y"].reshape(shard)
        valid = node_at[c] >= 0
        y[node_at[c][valid], 0] = yc[valid]
    return y


# revision 31
# speedup vs baseline: 1.0502x; 1.0186x over previous
"""BrainGCN kernel for 8 Trainium2 NeuronCores (Bass/Tile).

Strategy (v2):
- Nodes partitioned across 8 cores (degree-sorted snake deal), shard=6272
  locals per core (49 chunks of 128). Each node is assigned to one of two
  "halves" (table windows) with a greedy per-destination balance pass so that
  every destination's in-edges split ~evenly between halves; this keeps the
  round-padded slot structure tight (~1.1x instead of 1.45x).
- conv1: host pre-expands x*dinv into per-edge-slot columns (bf16,
  feature-major, round-major within each half's chunk group). The device
  streams slabs and accumulates directly in PSUM: one [64,128] PSUM region
  per chunk, matmuls accumulate rounds (start on k==0), so no DVE adds.
  h1 = tanh(psum*dinv + b1) read out per bank.
- z2 = (h1*dinv) @ W2 per chunk (node-major), converted to bf16 and
  AllGathered per half as soon as that half's chunks finish, overlapping the
  collective with the other half's conv1 streaming. Gathered bf16 tables are
  upconverted on device to fp32 tables for dma_gather (256B rows).
- conv2: big dma_gather groups (GBLK blocks = GBLK*128 indices each) on 4
  SWDGE queues; reduction on the PE via fp32r identity-matmuls accumulating
  into per-chunk [128,64] PSUM regions (1 cycle/row), h2 read out per bank.
- FC head: per-chunk PE transposes, feature-major matmuls, fused tanh+bias.

kernel(**inputs) takes FULL inputs, preprocesses + shards on host, compiles
and runs the SPMD program on cores 0..7, and reassembles the full output.
"""

import os
import warnings

warnings.filterwarnings("ignore")

import numpy as np
import ml_dtypes

from concourse import bacc, bass, mybir, tile
from concourse.masks import make_identity
import concourse.bass_utils as bass_utils

P = 128
NCORES = 8
GQ = int(os.environ.get("GCN_GQ", "4"))  # SWDGE queues for conv2 gathers
GBLK = int(os.environ.get("GCN_GBLK", "8"))  # blocks per dma_gather
SLAB = int(os.environ.get("GCN_SLAB", "2048"))  # conv1 stream columns per DMA
PE_REDUCE = bool(int(os.environ.get("GCN_PE_REDUCE", "1")))
AG8 = bool(int(os.environ.get("GCN_AG8", "0")))  # fp8 AllGather payload
X8 = bool(int(os.environ.get("GCN_X8", "0")))  # fp8 conv1 stream
F32R = bool(int(os.environ.get("GCN_F32R", "1")))  # fp32r PE reduce pipeline
ACTDMA = bool(int(os.environ.get("GCN_ACTDMA", "1")))  # allow ACT-engine DMAs
AGBF = bool(int(os.environ.get("GCN_AGBF", "1")))  # bf16 AllGather payload + upconvert
C1PSUM = bool(int(os.environ.get("GCN_C1PSUM", "1")))  # conv1 PSUM-region accumulate
ZSCALE = bool(int(os.environ.get("GCN_ZSCALE", "1")))  # fused dinv scale on z2 copy


# ---------------------------------------------------------------------------
# Host preprocessing
# ---------------------------------------------------------------------------

def _preprocess(x, edge_index):
    """Partition nodes, build slot structures and per-core input arrays."""
    N = x.shape[0]
    E = edge_index.shape[1]
    src = np.asarray(edge_index[0], dtype=np.int64)
    dst = np.asarray(edge_index[1], dtype=np.int64)

    shard = -(-N // (NCORES * P)) * P  # 6272
    nch = shard // P  # 49
    h0ch = (nch + 1) // 2  # 25
    h1ch = nch - h0ch  # 24
    H0 = h0ch * P  # 3200 positions per core in half 0
    H1 = h1ch * P  # 3072

    deg = 1 + np.bincount(dst, minlength=N)  # includes self-loop
    dinv = (1.0 / np.sqrt(deg)).astype(np.float32)

    counts = np.array([N // NCORES + (c < N % NCORES) for c in range(NCORES)])
    # phase A: global degree sort (desc), snake deal to cores
    order = np.argsort(-deg, kind="stable")
    core_of = np.empty(N, np.int32)
    taken = np.zeros(NCORES, np.int64)
    ci = 0
    direction = 1
    for v in order:
        for _ in range(NCORES):
            if taken[ci] < counts[ci]:
                break
            ci = (ci + direction) % NCORES
        core_of[v] = ci
        taken[ci] += 1
        ci += direction
        if ci == NCORES:
            ci, direction = NCORES - 1, -1
        elif ci == -1:
            ci, direction = 0, 1

    # out-adjacency (dests per source), for the balance pass
    sorder = np.argsort(src, kind="stable")
    src_ss = src[sorder]
    dst_ss = dst[sorder]
    ostarts = np.searchsorted(src_ss, np.arange(N))
    oends = np.searchsorted(src_ss, np.arange(N) + 1)

    # greedy balanced half assignment: process nodes in degree-desc order;
    # put v in the half where its out-destinations currently have fewer
    # sources, respecting per-(core,half) capacity (one pad row reserved).
    d0 = np.zeros(N, np.int32)
    d1 = np.zeros(N, np.int32)
    n0 = np.zeros(NCORES, np.int64)
    n1 = np.zeros(NCORES, np.int64)
    cap0, cap1 = H0 - 1, H1 - 1
    half_of = np.empty(N, np.int8)
    for v in order:
        c = core_of[v]
        D = dst_ss[ostarts[v]:oends[v]]
        s0 = int(d0[D].sum()) + int(d0[v])  # self-loop dest is v itself
        s1 = int(d1[D].sum()) + int(d1[v])
        h = 0 if s0 <= s1 else 1
        if h == 0 and n0[c] >= cap0:
            h = 1
        elif h == 1 and n1[c] >= cap1:
            h = 0
        half_of[v] = h
        if h == 0:
            np.add.at(d0, D, 1)
            d0[v] += 1
            n0[c] += 1
        else:
            np.add.at(d1, D, 1)
            d1[v] += 1
            n1[c] += 1
    assert (n0 <= cap0).all() and (n1 <= cap1).all()

    # refinement sweeps: re-assign each node to the half that best balances
    # its destinations' source counts (capacity-respecting)
    for _ in range(3):
        moved = 0
        for v in order:
            c = core_of[v]
            D = dst_ss[ostarts[v]:oends[v]]
            h_cur = half_of[v]
            # counts with v removed
            if h_cur == 0:
                np.add.at(d0, D, -1)
                d0[v] -= 1
                n0[c] -= 1
            else:
                np.add.at(d1, D, -1)
                d1[v] -= 1
                n1[c] -= 1
            s0 = int(d0[D].sum()) + int(d0[v])
            s1 = int(d1[D].sum()) + int(d1[v])
            h = 0 if s0 <= s1 else 1
            if h == 0 and n0[c] >= cap0:
                h = 1
            elif h == 1 and n1[c] >= cap1:
                h = 0
            if h != h_cur:
                moved += 1
            half_of[v] = h
            if h == 0:
                np.add.at(d0, D, 1)
                d0[v] += 1
                n0[c] += 1
            else:
                np.add.at(d1, D, 1)
                d1[v] += 1
                n1[c] += 1
        if moved == 0:
            break

    # phase B: positions within each (core, half) by total degree desc
    pos_of = np.full(N, -1, np.int64)
    for c in range(NCORES):
        for h, base in ((0, 0), (1, H0)):
            mask = (core_of == c) & (half_of == h)
            mem = np.nonzero(mask)[0]
            key = np.lexsort((-(d0[mem] if h == 0 else d1[mem]), -deg[mem]))
            pos_of[mem[key]] = base + np.arange(len(mem))

    # global half-table rows
    grow_h = np.where(
        half_of == 0,
        core_of.astype(np.int64) * H0 + pos_of,
        core_of.astype(np.int64) * H1 + (pos_of - H0),
    )

    # per-node in-edge src lists
    eorder = np.argsort(dst, kind="stable")
    dst_s = dst[eorder]
    src_s = src[eorder]
    starts = np.searchsorted(dst_s, np.arange(N))
    ends = np.searchsorted(dst_s, np.arange(N) + 1)

    # round counts per chunk (global max over cores)
    # conv1: total degree; conv2: per-source-half degree of the dest
    Ktg = np.zeros(nch, np.int32)
    K0g = np.zeros(nch, np.int32)
    K1g = np.zeros(nch, np.int32)
    ch_of = (pos_of // P).astype(np.int32)
    np.maximum.at(Ktg, ch_of, deg.astype(np.int32))
    np.maximum.at(K0g, ch_of, d0)
    np.maximum.at(K1g, ch_of, d1)
    assert (K0g >= 1).all() and (K1g >= 1).all() and (Ktg >= 1).all()

    # conv1 block list: half-group-major, then round-major
    def round_major(Karr, chs):
        kmax = int(Karr[chs].max()) if len(chs) else 0
        blocks = []
        for k in range(kmax):
            for ch in chs:
                if Karr[ch] > k:
                    blocks.append((k, ch))
        return blocks

    blocks1_g0 = round_major(Ktg, list(range(h0ch)))
    blocks1_g1 = round_major(Ktg, list(range(h0ch, nch)))
    blocks1 = blocks1_g0 + blocks1_g1
    nb1_g0 = len(blocks1_g0)
    S1 = len(blocks1) * P

    # conv2 block lists per source half, round-major over all chunks
    blocks2 = {
        0: round_major(K0g, list(range(nch))),
        1: round_major(K1g, list(range(nch))),
    }

    # conv2 gather groups (<= GBLK blocks each, within one source half)
    groups = []
    for half in (0, 1):
        blks = blocks2[half]
        for i in range(0, len(blks), GBLK):
            groups.append((half, blks[i : i + GBLK]))

    tot2 = (len(blocks2[0]) + len(blocks2[1])) * P
    per_core_work = (E + N) / NCORES
    print(
        f"[pre] shard={shard} conv1 slots={S1} ({S1/per_core_work:.3f}x) "
        f"conv2 slots={tot2} ({tot2/per_core_work:.3f}x) groups={len(groups)}"
    )

    node_at = np.full((NCORES, shard), -1, np.int64)
    node_at[core_of, pos_of] = np.arange(N)

    # PE reduce segments for conv2: runs of consecutive ch at the same k
    # within one PSUM bank. PSUM accumulation groups are bank-granular
    # (ZERO_REGION=2KB): exactly one start (first segment touching the bank,
    # which zeroes the whole bank) and one stop (last segment) per bank.
    group_segs = []
    for half, blks in groups:
        segs = []
        i = 0
        while i < len(blks):
            k, ch = blks[i]
            r = 1
            while (
                i + r < len(blks)
                and blks[i + r] == (k, ch + r)
                and (ch + r) // 8 == ch // 8
            ):
                r += 1
            segs.append([i, ch // 8, (ch % 8) * 64, r, False, False])
            i += r
        group_segs.append(segs)
    seen_bank = set()
    last_seg_of_bank = {}
    for gi, segs in enumerate(group_segs):
        for si_, seg in enumerate(segs):
            b = seg[1]
            if b not in seen_bank:
                seen_bank.add(b)
                seg[4] = True
            last_seg_of_bank[b] = (gi, si_)
    for b, (gi, si_) in last_seg_of_bank.items():
        group_segs[gi][si_][5] = True

    # conv1 PE segments: per 512-col matmul window of each slab we need runs
    # of consecutive ch within one conv1 PSUM bank (4 regions of [64,128]).
    # Built on the fly in _build from blocks1.

    xdt_np = (
        ml_dtypes.float8_e4m3fn
        if os.environ.get("GCN_X8", "0") == "1"
        else ml_dtypes.bfloat16
    )
    xs = x.astype(np.float32) * dinv[:, None]
    xsT = np.ascontiguousarray(xs.T).astype(xdt_np)  # [128, N]

    # per-core slot source arrays
    b1_of = {}
    for i, (k, ch) in enumerate(blocks1):
        b1_of[(k, ch)] = i
    b2_of = {0: {}, 1: {}}
    for h in (0, 1):
        for i, (k, ch) in enumerate(blocks2[h]):
            b2_of[h][(k, ch)] = i

    per_core = []
    for c in range(NCORES):
        src1 = np.full((len(blocks1), P), -1, np.int64)
        idx2 = {
            0: np.full((len(blocks2[0]), P), -1, np.int64),
            1: np.full((len(blocks2[1]), P), -1, np.int64),
        }
        zero_row = {0: c * H0 + H0 - 1, 1: c * H1 + H1 - 1}
        for pos in range(shard):
            v = node_at[c, pos]
            if v < 0:
                continue
            ch, p = pos // P, pos % P
            e0 = src_s[starts[v] : ends[v]]
            halves = half_of[e0]
            l0 = e0[halves == 0]
            l1 = e0[halves == 1]
            if half_of[v] == 0:
                l0 = np.concatenate(([v], l0))
            else:
                l1 = np.concatenate(([v], l1))
            ltot = np.concatenate((l0, l1))
            for k in range(len(ltot)):
                src1[b1_of[(k, ch)], p] = ltot[k]
            for h, lh in ((0, l0), (1, l1)):
                for k in range(len(lh)):
                    idx2[h][b2_of[h][(k, ch)], p] = grow_h[lh[k]]

        flat1 = src1.reshape(-1)
        x_exp = np.zeros((P, S1), dtype=xdt_np)
        valid = flat1 >= 0
        x_exp[:, valid] = xsT[:, flat1[valid]]

        slabs = []
        for gi, (half, blks) in enumerate(groups):
            idxs = np.empty((len(blks), P), np.int64)
            for j, (k, ch) in enumerate(blks):
                row = idx2[half][b2_of[half][(k, ch)]]
                idxs[j] = np.where(row >= 0, row, zero_row[half])
            flat = idxs.reshape(-1)
            assert flat.max() < 32768, flat.max()
            S = len(flat) // 16
            wrapped = flat.reshape(S, 16).T.astype(np.int16)  # [16, S]
            slabs.append(wrapped)
        idx_cat = np.concatenate(slabs, axis=1)
        idx_rep = np.tile(idx_cat, (8, 1))  # [128, sum S]

        dinv_loc = np.zeros(shard, np.float32)
        valid_pos = node_at[c] >= 0
        dinv_loc[valid_pos] = dinv[node_at[c][valid_pos]]
        dinv_fm = np.tile(dinv_loc[None, :], (64, 1)).astype(ml_dtypes.bfloat16)
        dinv_nm = dinv_loc.reshape(nch, P).T.astype(np.float32).copy()  # [128,nch]

        per_core.append(
            dict(x_exp=x_exp, idx=idx_rep, dinv_fm=dinv_fm, dinv_nm=dinv_nm)
        )

    struct = dict(
        N=N,
        shard=shard,
        nch=nch,
        h0ch=h0ch,
        h1ch=h1ch,
        H0=H0,
        H1=H1,
        blocks1=blocks1,
        nb1_g0=nb1_g0,
        groups=groups,
        group_segs=group_segs,
        blocks2=blocks2,
        S1=S1,
        node_at=node_at,
        idx_cols=per_core[0]["idx"].shape[1],
        K1g=K1g,
    )
    return struct, per_core, dinv


# ---------------------------------------------------------------------------
# Program builder
# ---------------------------------------------------------------------------

def _build(st, weights, n_passes=1):
    """Build the SPMD Bass program."""
    shard, nch = st["shard"], st["nch"]
    S1 = st["S1"]
    blocks1 = st["blocks1"]
    nb1_g0 = st["nb1_g0"]
    groups = st["groups"]
    group_segs = st["group_segs"]
    H0, H1 = st["H0"], st["H1"]
    h0ch, h1ch = st["h0ch"], st["h1ch"]
    bf16 = mybir.dt.bfloat16
    f32 = mybir.dt.float32
    f32r = mybir.dt.float32r
    tdt = f32r if (F32R and PE_REDUCE) else f32
    agdt = (mybir.dt.float8e4 if AG8 else bf16) if AGBF else tdt
    xdt = mybir.dt.float8e4 if X8 else bf16

    fb2 = float(np.asarray(weights["fc_b2"]).reshape(-1)[0])

    nc = bacc.Bacc(
        "TRN2",
        target_bir_lowering=False,
        debug=False,
        enable_asserts=False,
        num_devices=NCORES,
        num_swdge_queues=GQ,
    )

    x_exp_in = nc.dram_tensor("x_exp", [P, S1], xdt, kind="ExternalInput")
    idx_in = nc.dram_tensor(
        "idx2", [P, st["idx_cols"]], mybir.dt.int16, kind="ExternalInput"
    )
    dinv_fm_in = nc.dram_tensor("dinv_fm", [64, shard], bf16, kind="ExternalInput")
    dinv_nm_in = nc.dram_tensor("dinv_nm", [P, nch], f32, kind="ExternalInput")
    w1_in = nc.dram_tensor("w1", [P, 64], xdt, kind="ExternalInput")
    w2_in = nc.dram_tensor("w2", [64, 64], bf16, kind="ExternalInput")
    fw1_in = nc.dram_tensor("fw1", [64, 32], bf16, kind="ExternalInput")
    fw2_in = nc.dram_tensor("fw2", [32, 1], bf16, kind="ExternalInput")
    b1_in = nc.dram_tensor("b1c", [64, 1], f32, kind="ExternalInput")
    b2e_in = nc.dram_tensor("b2e", [P, 64], f32, kind="ExternalInput")
    fb1_in = nc.dram_tensor("fb1c", [32, 1], f32, kind="ExternalInput")
    y_out = nc.dram_tensor("y", [1, shard], f32, kind="ExternalOutput")

    with tile.TileContext(nc) as tc:
        with (
            tc.tile_pool(name="const", bufs=1) as constp,
            tc.tile_pool(name="big", bufs=1) as bigp,
            tc.tile_pool(name="xslab", bufs=5 if C1PSUM else 2) as xslabp,
            tc.tile_pool(name="gstage", bufs=3) as gstagep,
            tc.tile_pool(name="upc", bufs=2) as upcp,
            tc.tile_pool(name="psum", bufs=8, space="PSUM") as psump,
            tc.tile_pool(name="small", bufs=2) as smallp,
            tc.tile_pool(name="dram", bufs=1, space="DRAM") as dramp,
        ):
            # constants
            w1_sb = constp.tile([P, 64], xdt, name="w1_sb")
            nc.sync.dma_start(out=w1_sb[:], in_=w1_in.ap())
            w2_sb = constp.tile([64, 64], bf16, name="w2_sb")
            nc.sync.dma_start(out=w2_sb[:], in_=w2_in.ap())
            fw1_sb = constp.tile([64, 32], bf16, name="fw1_sb")
            nc.sync.dma_start(out=fw1_sb[:], in_=fw1_in.ap())
            fw2_sb = constp.tile([32, 1], bf16, name="fw2_sb")
            nc.sync.dma_start(out=fw2_sb[:], in_=fw2_in.ap())
            b1_sb = constp.tile([64, 1], f32, name="b1_sb")
            nc.sync.dma_start(out=b1_sb[:], in_=b1_in.ap())
            b2e_sb = constp.tile([P, 64], f32, name="b2e_sb")
            nc.sync.dma_start(out=b2e_sb[:], in_=b2e_in.ap())
            fb1_sb = constp.tile([32, 1], f32, name="fb1_sb")
            nc.sync.dma_start(out=fb1_sb[:], in_=fb1_in.ap())
            dinv_fm = constp.tile([64, shard], bf16, name="dinv_fm_sb")
            nc.sync.dma_start(out=dinv_fm[:], in_=dinv_fm_in.ap())
            dinv_nm = constp.tile([P, nch], f32, name="dinv_nm_sb")
            nc.sync.dma_start(out=dinv_nm[:], in_=dinv_nm_in.ap())
            ident = constp.tile([P, P], f32, name="ident")
            make_identity(nc, ident[:])
            idx_sb = constp.tile([P, st["idx_cols"]], mybir.dt.int16, name="idx_sb")
            if F32R and PE_REDUCE:
                ident_r = constp.tile([P, P], f32r, name="ident_r")
                nc.vector.tensor_copy(out=ident_r[:], in_=ident[:])
            else:
                ident_r = ident

            # DRAM tables
            t0h = dramp.tile([NCORES * H0, 64], agdt, name="t0h", addr_space="Shared")
            t1h = dramp.tile([NCORES * H1, 64], agdt, name="t1h", addr_space="Shared")
            if AGBF:
                t0 = dramp.tile([NCORES * H0, 64], tdt, name="t0")
                t1 = dramp.tile([NCORES * H1, 64], tdt, name="t1")
            else:
                t0, t1 = t0h, t1h
            ag0_in = dramp.tile([H0, 64], agdt, name="ag0i")
            ag1_in = dramp.tile([H1, 64], agdt, name="ag1i")

            h1s = bigp.tile([64, shard], bf16, name="h1s", tag="fm")
            acc1 = None
            if not C1PSUM:
                acc1 = bigp.tile([64, shard], f32, name="acc1", tag="acc1")

            # per-chunk conv1 round counts from blocks1
            _KT = {}
            for k, ch in blocks1:
                _KT[ch] = max(_KT.get(ch, 0), k + 1)

            # ------------- conv1 (per half group) + z2 + AllGather ----------
            def conv1_half(hg):
                b_lo = 0 if hg == 0 else nb1_g0
                b_hi = nb1_g0 if hg == 0 else len(blocks1)
                ch_lo = 0 if hg == 0 else h0ch
                nch_g = h0ch if hg == 0 else h1ch
                # PSUM regions: 4 chunks of [64,128] per bank
                nbank = -(-nch_g // 4)
                regs = []
                if C1PSUM:
                    for b in range(nbank):
                        rg = psump.tile(
                            [P, 512], f32, tag="ps", bufs=8, name=f"c1r_{hg}_{b}"
                        )
                        regs.append(rg)

                c_lo, c_hi = b_lo * P, b_hi * P
                n_slabs = -(-(c_hi - c_lo) // SLAB)
                # pre-enumerate matmul segments: (slab, i, r, bank, col)
                seg_list = []
                for si in range(n_slabs):
                    s0 = c_lo + si * SLAB
                    s1_ = min(c_hi, s0 + SLAB)
                    i = s0 // P
                    bend = s1_ // P
                    while i < bend:
                        k, ch = blocks1[i]
                        lch = ch - ch_lo
                        r = 1
                        while (
                            i + r < bend
                            and blocks1[i + r] == (k, ch + r)
                            and (lch + r) // 4 == lch // 4
                        ):
                            r += 1
                        seg_list.append([si, i, r, lch // 4, (lch % 4) * P])
                        i += r
                seen_b = set()
                last_of_b = {}
                flags = []
                for j, (si, i, r, b, col) in enumerate(seg_list):
                    st_f = b not in seen_b
                    seen_b.add(b)
                    last_of_b[b] = j
                    flags.append([st_f, False])
                for b, j in last_of_b.items():
                    flags[j][1] = True

                jseg = 0
                for si in range(n_slabs):
                    s0 = c_lo + si * SLAB
                    s1_ = min(c_hi, s0 + SLAB)
                    xsl = xslabp.tile([P, SLAB], xdt, tag="xsl", name=f"xsl_{hg}_{si}")
                    eng = nc.sync if (si % 2 == 0 or not ACTDMA) else nc.scalar
                    eng.dma_start(out=xsl[:, : s1_ - s0], in_=x_exp_in.ap()[:, s0:s1_])
                    if C1PSUM:
                        while jseg < len(seg_list) and seg_list[jseg][0] == si:
                            _, i, r, b, col = seg_list[jseg]
                            st_f, sp_f = flags[jseg]
                            nc.tensor.matmul(
                                regs[b][:64, col : col + r * P],
                                lhsT=w1_sb[:],
                                rhs=xsl[:, (i * P - s0) : (i * P - s0) + r * P],
                                start=st_f,
                                stop=sp_f,
                            )
                            jseg += 1
                    else:
                        for m0 in range(s0, s1_, 512):
                            m1 = min(s1_, m0 + 512)
                            pt = psump.tile(
                                [P, 512], f32, tag="ps", bufs=8, name=f"ps1_{hg}_{m0}"
                            )
                            nc.tensor.matmul(
                                pt[:64, : m1 - m0],
                                lhsT=w1_sb[:],
                                rhs=xsl[:, m0 - s0 : m1 - s0],
                                start=True,
                                stop=True,
                            )
                            b0, bend2 = m0 // P, m1 // P
                            i = b0
                            while i < bend2:
                                k, ch = blocks1[i]
                                r = 1
                                while i + r < bend2 and blocks1[i + r] == (k, ch + r):
                                    r += 1
                                if k == 0:
                                    nc.scalar.copy(
                                        out=acc1[:, ch * P : ch * P + r * P],
                                        in_=pt[:64, (i - b0) * P : (i - b0 + r) * P],
                                    )
                                else:
                                    nc.vector.tensor_add(
                                        acc1[:, ch * P : ch * P + r * P],
                                        acc1[:, ch * P : ch * P + r * P],
                                        pt[:64, (i - b0) * P : (i - b0 + r) * P],
                                    )
                                i += r

                # h1 = tanh(acc*dinv + b1), per bank readout
                cols0 = ch_lo * P
                for b in range(nbank):
                    w = min(512, (nch_g - b * 4) * P)
                    a0 = cols0 + b * 512
                    nc.vector.tensor_mul(
                        h1s[:, a0 : a0 + w],
                        regs[b][:64, :w] if C1PSUM else acc1[:, a0 : a0 + w],
                        dinv_fm[:, a0 : a0 + w],
                    )
                gw = nch_g * P
                nc.scalar.activation(
                    h1s[:, cols0 : cols0 + gw],
                    h1s[:, cols0 : cols0 + gw],
                    mybir.ActivationFunctionType.Tanh,
                    bias=b1_sb[:, :1],
                )

                # z2 (node-major bf16) for this half's chunks
                z2st = smallp.tile(
                    [P, nch_g * 64], agdt, tag="z2st", name=f"z2st_{hg}"
                )
                for j in range(nch_g):
                    ch = ch_lo + j
                    pz = psump.tile(
                        [P, 512], f32, tag="ps", bufs=8, name=f"pz_{hg}_{j}"
                    )
                    nc.tensor.matmul(
                        pz[:, :64],
                        lhsT=h1s[:, ch * P : (ch + 1) * P],
                        rhs=w2_sb[:],
                        start=True,
                        stop=True,
                    )
                    # z2 = (h1 @ W2) * dinv (per-node row scale, fused here)
                    if not ZSCALE:
                        nc.vector.tensor_scalar_mul(
                            z2st[:, j * 64 : (j + 1) * 64],
                            pz[:, :64],
                            dinv_nm[:, ch : ch + 1],
                        )
                    elif j % 2 == 0:
                        nc.scalar.activation(
                            z2st[:, j * 64 : (j + 1) * 64],
                            pz[:, :64],
                            mybir.ActivationFunctionType.Copy,
                            scale=dinv_nm[:, ch : ch + 1],
                        )
                    else:
                        nc.vector.tensor_scalar_mul(
                            z2st[:, j * 64 : (j + 1) * 64],
                            pz[:, :64],
                            dinv_nm[:, ch : ch + 1],
                        )

                ag_in = ag0_in if hg == 0 else ag1_in
                th = t0h if hg == 0 else t1h
                (nc.scalar if ACTDMA else nc.sync).dma_start(
                    out=ag_in[:].rearrange("(c p) f -> p c f", p=P),
                    in_=z2st[:].rearrange("p (c f) -> p c f", f=64),
                )
                nc.gpsimd.collective_compute(
                    "AllGather",
                    mybir.AluOpType.bypass,
                    replica_groups=[list(range(NCORES))],
                    ins=[ag_in.opt()],
                    outs=[th.opt()],
                )

            # upconvert one bf16 half-table to fp32, in pieces.
            # partition-major view: partition p holds table rows
            # [p*a_tot, (p+1)*a_tot) so each DMA is 128 contiguous runs.
            def upconvert(th, tf, rows, who, act_ok, gate=None):
                # act_ok=False keeps the ACT queue clear (so a later ag DMA
                # is not stuck behind pieces that wait on this AllGather).
                # gate: tiny DRAM AP whose write must precede this phase —
                # read one element into the staging tiles so the scheduler
                # cannot hoist these pieces ahead of the gate's producer.
                a_tot = rows // P
                PIECE = -(-a_tot // 8)
                for pi, a0 in enumerate(range(0, a_tot, PIECE)):
                    a1 = min(a_tot, a0 + PIECE)
                    w = (a1 - a0) * 64
                    ub = upcp.tile([P, PIECE * 64], agdt, tag="ub", name=f"ub_{who}_{pi}")
                    uf = upcp.tile([P, PIECE * 64], tdt, tag="uf", name=f"uf_{who}_{pi}")
                    eng = nc.scalar if (ACTDMA and act_ok and pi % 2 == 1) else nc.sync
                    if gate is not None and pi < 2:
                        eng.dma_start(out=ub[0:1, 0:1], in_=gate)
                        eng.dma_start(out=uf[0:1, 0:1].bitcast(agdt)[:, 0:1], in_=gate)
                    eng.dma_start(
                        out=ub[:, :w].rearrange("p (a f) -> p a f", f=64),
                        in_=th[:].rearrange("(p a) f -> p a f", p=P)[:, a0:a1, :],
                    )
                    if act_ok and pi % 2 == 1:
                        nc.scalar.copy(out=uf[:, :w], in_=ub[:, :w])
                    else:
                        nc.vector.tensor_copy(out=uf[:, :w], in_=ub[:, :w])
                    eng.dma_start(
                        out=tf[:].rearrange("(p a) f -> p a f", p=P)[:, a0:a1, :],
                        in_=uf[:, :w].rearrange("p (a f) -> p a f", f=64),
                    )

            # ------------- conv2 gathers + PE reduce ------------------------
            _gctr = [0]

            def conv2_half(sh, regs2):
                tab = t0 if sh == 0 else t1
                icol = _icol_of[sh]
                for gi, (half, blks) in enumerate(groups):
                    if half != sh:
                        continue
                    nb = len(blks)
                    nidx = nb * P
                    S = nidx // 16
                    stg = gstagep.tile(
                        [P, GBLK * 64], tdt, tag="stg", name=f"stg_{sh}_{gi}"
                    )
                    nc.gpsimd.dma_gather(
                        stg[:, : nb * 64].rearrange("p (b d) -> p b d", d=64),
                        tab[:],
                        idx_sb[:, icol : icol + S],
                        nidx,
                        nidx,
                        64,
                        queue_num=_gctr[0] % GQ,
                    )
                    _gctr[0] += 1
                    icol += S
                    for (i, bank, pcol, r, fstart, fstop) in group_segs[gi]:
                        if PE_REDUCE:
                            nc.tensor.matmul(
                                regs2[bank][:, pcol : pcol + r * 64],
                                lhsT=ident_r[:],
                                rhs=stg[:, i * 64 : (i + r) * 64],
                                start=fstart,
                                stop=fstop,
                            )
                        else:
                            k, ch = blks[i]
                            a0 = ch * 64
                            if fstart:
                                nc.scalar.copy(
                                    out=acc2[:, a0 : a0 + r * 64],
                                    in_=stg[:, i * 64 : (i + r) * 64],
                                )
                            else:
                                nc.vector.tensor_add(
                                    acc2[:, a0 : a0 + r * 64],
                                    acc2[:, a0 : a0 + r * 64],
                                    stg[:, i * 64 : (i + r) * 64],
                                )

            # precompute idx column offsets per source half
            _icol_of = {0: 0, 1: 0}
            icol = 0
            for gi, (half, blks) in enumerate(groups):
                if half == 1 and _icol_of[1] == 0:
                    _icol_of[1] = icol
                icol += len(blks) * P // 16

            # =================== emission order =============================
            conv1_half(0)
            # idx table is not needed until the conv2 gathers; load it in the
            # SP idle window so it does not delay the first conv1 slab
            nc.sync.dma_start(out=idx_sb[:], in_=idx_in.ap())
            conv1_half(1)
            if AGBF:
                upconvert(t0h, t0, NCORES * H0, "t0", act_ok=False, gate=ag1_in[0:1, 0:1])

            if PE_REDUCE:
                regs2 = [
                    psump.tile([P, 512], f32, tag="ps", bufs=8, name=f"c2r_{b}")
                    for b in range(-(-nch // 8))
                ]
                acc2 = None
            else:
                regs2 = None
                acc2 = bigp.tile([P, nch * 64], f32, name="acc2", tag="acc2")

            conv2_half(0, regs2)
            if AGBF:
                upconvert(t1h, t1, NCORES * H1, "t1", act_ok=True)
            conv2_half(1, regs2)

            # h2 = tanh(acc2*dinv_nm + b2) node-major
            h2 = bigp.tile([P, nch * 64], f32, name="h2", tag="h2")
            for ch in range(nch):
                if PE_REDUCE:
                    src_ap = regs2[ch // 8][:, (ch % 8) * 64 : (ch % 8) * 64 + 64]
                else:
                    src_ap = acc2[:, ch * 64 : (ch + 1) * 64]
                nc.vector.scalar_tensor_tensor(
                    out=h2[:, ch * 64 : (ch + 1) * 64],
                    in0=src_ap,
                    scalar=dinv_nm[:, ch : ch + 1],
                    in1=b2e_sb[:],
                    op0=mybir.AluOpType.mult,
                    op1=mybir.AluOpType.add,
                )
            nc.scalar.activation(h2[:], h2[:], mybir.ActivationFunctionType.Tanh)

            # ------------- FC head -----------------------------------------
            h2fm = bigp.tile([64, shard], bf16, name="h2fm", tag="fm")
            for ch in range(nch):
                ptr = psump.tile([P, 512], f32, tag="ps", bufs=8, name=f"pst_{ch}")
                nc.tensor.transpose(
                    out=ptr[:64, :P],
                    in_=h2[:, ch * 64 : (ch + 1) * 64],
                    identity=ident[:],
                )
                if ch % 2 == 0:
                    nc.scalar.copy(out=h2fm[:, ch * P : (ch + 1) * P], in_=ptr[:64, :P])
                else:
                    nc.vector.tensor_copy(out=h2fm[:, ch * P : (ch + 1) * P], in_=ptr[:64, :P])

            h3 = bigp.tile([32, shard], bf16, name="h3", tag="h3")
            for m0 in range(0, shard, 512):
                m1 = min(shard, m0 + 512)
                pf = psump.tile([P, 512], f32, tag="ps", bufs=8, name=f"psf_{m0}")
                nc.tensor.matmul(
                    pf[:32, : m1 - m0], lhsT=fw1_sb[:], rhs=h2fm[:, m0:m1],
                    start=True, stop=True,
                )
                nc.scalar.activation(
                    h3[:, m0:m1],
                    pf[:32, : m1 - m0],
                    mybir.ActivationFunctionType.Tanh,
                    bias=fb1_sb[:, :1],
                )
            ysb = smallp.tile([1, shard], f32, tag="ysb", bufs=1, name="ysb")
            for m0 in range(0, shard, 512):
                m1 = min(shard, m0 + 512)
                pg = psump.tile([P, 512], f32, tag="ps", bufs=8, name=f"psg_{m0}")
                nc.tensor.matmul(
                    pg[:1, : m1 - m0], lhsT=fw2_sb[:], rhs=h3[:, m0:m1],
                    start=True, stop=True,
                )
                if (m0 // 512) % 2 == 0:
                    nc.scalar.activation(
                        ysb[:, m0:m1],
                        pg[:1, : m1 - m0],
                        mybir.ActivationFunctionType.Copy,
                        bias=fb2,
                    )
                else:
                    nc.vector.tensor_scalar_add(ysb[:, m0:m1], pg[:1, : m1 - m0], fb2)
            nc.sync.dma_start(out=y_out.ap(), in_=ysb[:])

    nc.compile()
    return nc


# ---------------------------------------------------------------------------
# Entry point
# ---------------------------------------------------------------------------

def _in_maps(st, per_core, weights):
    w1dt = (
        ml_dtypes.float8_e4m3fn
        if os.environ.get("GCN_X8", "0") == "1"
        else ml_dtypes.bfloat16
    )
    w1 = np.asarray(weights["conv_w1"], np.float32).astype(w1dt)
    w2 = np.asarray(weights["conv_w2"], np.float32).astype(ml_dtypes.bfloat16)
    fw1 = np.asarray(weights["fc_w1"], np.float32).astype(ml_dtypes.bfloat16)
    fw2 = np.asarray(weights["fc_w2"], np.float32).astype(ml_dtypes.bfloat16)
    b1 = np.asarray(weights["conv_b1"], np.float32).reshape(64, 1)
    b2e = np.tile(np.asarray(weights["conv_b2"], np.float32)[None, :], (P, 1))
    fb1 = np.asarray(weights["fc_b1"], np.float32).reshape(32, 1)
    maps = []
    for c in range(NCORES):
        pc = per_core[c]
        maps.append(
            {
                "x_exp": pc["x_exp"],
                "idx2": pc["idx"],
                "dinv_fm": pc["dinv_fm"],
                "dinv_nm": pc["dinv_nm"],
                "w1": np.ascontiguousarray(w1),
                "w2": np.ascontiguousarray(w2),
                "fw1": np.ascontiguousarray(fw1),
                "fw2": np.ascontiguousarray(fw2),
                "b1c": b1,
                "b2e": b2e,
                "fb1c": fb1,
            }
        )
    return maps


def kernel(**inputs):
    x = np.asarray(inputs["x"], np.float32)
    edge_index = np.asarray(inputs["edge_index"])
    weights = {
        k: np.asarray(inputs[k], np.float32)
        for k in (
            "conv_w1",
            "conv_b1",
            "conv_w2",
            "conv_b2",
            "fc_w1",
            "fc_b1",
            "fc_w2",
            "fc_b2",
        )
    }
    st, per_core, dinv = _preprocess(x, edge_index)
    nc = _build(st, weights, n_passes=1)
    maps = _in_maps(st, per_core, weights)
    res = None
    for attempt in range(3):
        try:
            res = bass_utils.run_bass_kernel_spmd(
                nc, maps, core_ids=list(range(NCORES))
            )
            break
        except Exception as e:  # device wedge: retry
            if attempt == 2:
                raise
            print(f"[kernel] run attempt {attempt} failed ({e}); retrying")
    N, shard = st["N"], st["shard"]
    node_at = st["node_at"]
    y = np.empty((N, 1), np.float32)
    for c in range(NCORES):
        yc = res.results[c]["y"].reshape(shard)
        valid = node_at[c] >= 0
        y[node_at[c][valid], 0] = yc[valid]
    return y


# revision 32
# speedup vs baseline: 1.0559x; 1.0054x over previous
"""BrainGCN kernel for 8 Trainium2 NeuronCores (Bass/Tile).

Strategy (v2):
- Nodes partitioned across 8 cores (degree-sorted snake deal), shard=6272
  locals per core (49 chunks of 128). Each node is assigned to one of two
  "halves" (table windows) with a greedy per-destination balance pass so that
  every destination's in-edges split ~evenly between halves; this keeps the
  round-padded slot structure tight (~1.1x instead of 1.45x).
- conv1: host pre-expands x*dinv into per-edge-slot columns (bf16,
  feature-major, round-major within each half's chunk group). The device
  streams slabs and accumulates directly in PSUM: one [64,128] PSUM region
  per chunk, matmuls accumulate rounds (start on k==0), so no DVE adds.
  h1 = tanh(psum*dinv + b1) read out per bank.
- z2 = (h1*dinv) @ W2 per chunk (node-major), converted to bf16 and
  AllGathered per half as soon as that half's chunks finish, overlapping the
  collective with the other half's conv1 streaming. Gathered bf16 tables are
  upconverted on device to fp32 tables for dma_gather (256B rows).
- conv2: big dma_gather groups (GBLK blocks = GBLK*128 indices each) on 4
  SWDGE queues; reduction on the PE via fp32r identity-matmuls accumulating
  into per-chunk [128,64] PSUM regions (1 cycle/row), h2 read out per bank.
- FC head: per-chunk PE transposes, feature-major matmuls, fused tanh+bias.

kernel(**inputs) takes FULL inputs, preprocesses + shards on host, compiles
and runs the SPMD program on cores 0..7, and reassembles the full output.
"""

import os
import warnings

warnings.filterwarnings("ignore")

import numpy as np
import ml_dtypes

from concourse import bacc, bass, mybir, tile
from concourse.masks import make_identity
import concourse.bass_utils as bass_utils

P = 128
NCORES = 8
GQ = int(os.environ.get("GCN_GQ", "4"))  # SWDGE queues for conv2 gathers
GBLK = int(os.environ.get("GCN_GBLK", "8"))  # blocks per dma_gather
SLAB = int(os.environ.get("GCN_SLAB", "2048"))  # conv1 stream columns per DMA
PE_REDUCE = bool(int(os.environ.get("GCN_PE_REDUCE", "1")))
AG8 = bool(int(os.environ.get("GCN_AG8", "0")))  # fp8 AllGather payload
X8 = bool(int(os.environ.get("GCN_X8", "0")))  # fp8 conv1 stream
F32R = bool(int(os.environ.get("GCN_F32R", "1")))  # fp32r PE reduce pipeline
ACTDMA = bool(int(os.environ.get("GCN_ACTDMA", "1")))  # allow ACT-engine DMAs
AGBF = bool(int(os.environ.get("GCN_AGBF", "1")))  # bf16 AllGather payload + upconvert
C1PSUM = bool(int(os.environ.get("GCN_C1PSUM", "1")))  # conv1 PSUM-region accumulate
ZSCALE = bool(int(os.environ.get("GCN_ZSCALE", "1")))  # fused dinv scale on z2 copy


# ---------------------------------------------------------------------------
# Host preprocessing
# ---------------------------------------------------------------------------

def _preprocess(x, edge_index):
    """Partition nodes, build slot structures and per-core input arrays."""
    N = x.shape[0]
    E = edge_index.shape[1]
    src = np.asarray(edge_index[0], dtype=np.int64)
    dst = np.asarray(edge_index[1], dtype=np.int64)

    shard = -(-N // (NCORES * P)) * P  # 6272
    nch = shard // P  # 49
    h0ch = (nch + 1) // 2  # 25
    h1ch = nch - h0ch  # 24
    H0 = h0ch * P  # 3200 positions per core in half 0
    H1 = h1ch * P  # 3072

    deg = 1 + np.bincount(dst, minlength=N)  # includes self-loop
    dinv = (1.0 / np.sqrt(deg)).astype(np.float32)

    counts = np.array([N // NCORES + (c < N % NCORES) for c in range(NCORES)])
    # phase A: global degree sort (desc), snake deal to cores
    order = np.argsort(-deg, kind="stable")
    core_of = np.empty(N, np.int32)
    taken = np.zeros(NCORES, np.int64)
    ci = 0
    direction = 1
    for v in order:
        for _ in range(NCORES):
            if taken[ci] < counts[ci]:
                break
            ci = (ci + direction) % NCORES
        core_of[v] = ci
        taken[ci] += 1
        ci += direction
        if ci == NCORES:
            ci, direction = NCORES - 1, -1
        elif ci == -1:
            ci, direction = 0, 1

    # out-adjacency (dests per source), for the balance pass
    sorder = np.argsort(src, kind="stable")
    src_ss = src[sorder]
    dst_ss = dst[sorder]
    ostarts = np.searchsorted(src_ss, np.arange(N))
    oends = np.searchsorted(src_ss, np.arange(N) + 1)

    # greedy balanced half assignment: process nodes in degree-desc order;
    # put v in the half where its out-destinations currently have fewer
    # sources, respecting per-(core,half) capacity (one pad row reserved).
    d0 = np.zeros(N, np.int32)
    d1 = np.zeros(N, np.int32)
    n0 = np.zeros(NCORES, np.int64)
    n1 = np.zeros(NCORES, np.int64)
    cap0, cap1 = H0 - 1, H1 - 1
    half_of = np.empty(N, np.int8)
    for v in order:
        c = core_of[v]
        D = dst_ss[ostarts[v]:oends[v]]
        s0 = int(d0[D].sum()) + int(d0[v])  # self-loop dest is v itself
        s1 = int(d1[D].sum()) + int(d1[v])
        h = 0 if s0 <= s1 else 1
        if h == 0 and n0[c] >= cap0:
            h = 1
        elif h == 1 and n1[c] >= cap1:
            h = 0
        half_of[v] = h
        if h == 0:
            np.add.at(d0, D, 1)
            d0[v] += 1
            n0[c] += 1
        else:
            np.add.at(d1, D, 1)
            d1[v] += 1
            n1[c] += 1
    assert (n0 <= cap0).all() and (n1 <= cap1).all()

    # refinement sweeps: re-assign each node to the half that best balances
    # its destinations' source counts (capacity-respecting)
    for _ in range(3):
        moved = 0
        for v in order:
            c = core_of[v]
            D = dst_ss[ostarts[v]:oends[v]]
            h_cur = half_of[v]
            # counts with v removed
            if h_cur == 0:
                np.add.at(d0, D, -1)
                d0[v] -= 1
                n0[c] -= 1
            else:
                np.add.at(d1, D, -1)
                d1[v] -= 1
                n1[c] -= 1
            s0 = int(d0[D].sum()) + int(d0[v])
            s1 = int(d1[D].sum()) + int(d1[v])
            h = 0 if s0 <= s1 else 1
            if h == 0 and n0[c] >= cap0:
                h = 1
            elif h == 1 and n1[c] >= cap1:
                h = 0
            if h != h_cur:
                moved += 1
            half_of[v] = h
            if h == 0:
                np.add.at(d0, D, 1)
                d0[v] += 1
                n0[c] += 1
            else:
                np.add.at(d1, D, 1)
                d1[v] += 1
                n1[c] += 1
        if moved == 0:
            break

    # phase B: positions within each (core, half) by total degree desc
    pos_of = np.full(N, -1, np.int64)
    for c in range(NCORES):
        for h, base in ((0, 0), (1, H0)):
            mask = (core_of == c) & (half_of == h)
            mem = np.nonzero(mask)[0]
            key = np.lexsort((-(d0[mem] if h == 0 else d1[mem]), -deg[mem]))
            pos_of[mem[key]] = base + np.arange(len(mem))

    # global half-table rows
    grow_h = np.where(
        half_of == 0,
        core_of.astype(np.int64) * H0 + pos_of,
        core_of.astype(np.int64) * H1 + (pos_of - H0),
    )

    # per-node in-edge src lists
    eorder = np.argsort(dst, kind="stable")
    dst_s = dst[eorder]
    src_s = src[eorder]
    starts = np.searchsorted(dst_s, np.arange(N))
    ends = np.searchsorted(dst_s, np.arange(N) + 1)

    # round counts per chunk (global max over cores)
    # conv1: total degree; conv2: per-source-half degree of the dest
    Ktg = np.zeros(nch, np.int32)
    K0g = np.zeros(nch, np.int32)
    K1g = np.zeros(nch, np.int32)
    ch_of = (pos_of // P).astype(np.int32)
    np.maximum.at(Ktg, ch_of, deg.astype(np.int32))
    np.maximum.at(K0g, ch_of, d0)
    np.maximum.at(K1g, ch_of, d1)
    assert (K0g >= 1).all() and (K1g >= 1).all() and (Ktg >= 1).all()

    # conv1 block list: half-group-major, then round-major
    def round_major(Karr, chs):
        kmax = int(Karr[chs].max()) if len(chs) else 0
        blocks = []
        for k in range(kmax):
            for ch in chs:
                if Karr[ch] > k:
                    blocks.append((k, ch))
        return blocks

    blocks1_g0 = round_major(Ktg, list(range(h0ch)))
    blocks1_g1 = round_major(Ktg, list(range(h0ch, nch)))
    blocks1 = blocks1_g0 + blocks1_g1
    nb1_g0 = len(blocks1_g0)
    S1 = len(blocks1) * P

    # conv2 block lists per source half, round-major over all chunks
    blocks2 = {
        0: round_major(K0g, list(range(nch))),
        1: round_major(K1g, list(range(nch))),
    }

    # conv2 gather groups (<= GBLK blocks each, within one source half)
    groups = []
    for half in (0, 1):
        blks = blocks2[half]
        for i in range(0, len(blks), GBLK):
            groups.append((half, blks[i : i + GBLK]))

    tot2 = (len(blocks2[0]) + len(blocks2[1])) * P
    per_core_work = (E + N) / NCORES
    print(
        f"[pre] shard={shard} conv1 slots={S1} ({S1/per_core_work:.3f}x) "
        f"conv2 slots={tot2} ({tot2/per_core_work:.3f}x) groups={len(groups)}"
    )

    node_at = np.full((NCORES, shard), -1, np.int64)
    node_at[core_of, pos_of] = np.arange(N)

    # PE reduce segments for conv2: runs of consecutive ch at the same k
    # within one PSUM bank. PSUM accumulation groups are bank-granular
    # (ZERO_REGION=2KB): exactly one start (first segment touching the bank,
    # which zeroes the whole bank) and one stop (last segment) per bank.
    group_segs = []
    for half, blks in groups:
        segs = []
        i = 0
        while i < len(blks):
            k, ch = blks[i]
            r = 1
            while (
                i + r < len(blks)
                and blks[i + r] == (k, ch + r)
                and (ch + r) // 8 == ch // 8
            ):
                r += 1
            segs.append([i, ch // 8, (ch % 8) * 64, r, False, False])
            i += r
        group_segs.append(segs)
    seen_bank = set()
    last_seg_of_bank = {}
    for gi, segs in enumerate(group_segs):
        for si_, seg in enumerate(segs):
            b = seg[1]
            if b not in seen_bank:
                seen_bank.add(b)
                seg[4] = True
            last_seg_of_bank[b] = (gi, si_)
    for b, (gi, si_) in last_seg_of_bank.items():
        group_segs[gi][si_][5] = True

    # conv1 PE segments: per 512-col matmul window of each slab we need runs
    # of consecutive ch within one conv1 PSUM bank (4 regions of [64,128]).
    # Built on the fly in _build from blocks1.

    xdt_np = (
        ml_dtypes.float8_e4m3fn
        if os.environ.get("GCN_X8", "0") == "1"
        else ml_dtypes.bfloat16
    )
    xs = x.astype(np.float32) * dinv[:, None]
    xsT = np.ascontiguousarray(xs.T).astype(xdt_np)  # [128, N]

    # per-core slot source arrays
    b1_of = {}
    for i, (k, ch) in enumerate(blocks1):
        b1_of[(k, ch)] = i
    b2_of = {0: {}, 1: {}}
    for h in (0, 1):
        for i, (k, ch) in enumerate(blocks2[h]):
            b2_of[h][(k, ch)] = i

    per_core = []
    for c in range(NCORES):
        src1 = np.full((len(blocks1), P), -1, np.int64)
        idx2 = {
            0: np.full((len(blocks2[0]), P), -1, np.int64),
            1: np.full((len(blocks2[1]), P), -1, np.int64),
        }
        zero_row = {0: c * H0 + H0 - 1, 1: c * H1 + H1 - 1}
        for pos in range(shard):
            v = node_at[c, pos]
            if v < 0:
                continue
            ch, p = pos // P, pos % P
            e0 = src_s[starts[v] : ends[v]]
            halves = half_of[e0]
            l0 = e0[halves == 0]
            l1 = e0[halves == 1]
            if half_of[v] == 0:
                l0 = np.concatenate(([v], l0))
            else:
                l1 = np.concatenate(([v], l1))
            ltot = np.concatenate((l0, l1))
            for k in range(len(ltot)):
                src1[b1_of[(k, ch)], p] = ltot[k]
            for h, lh in ((0, l0), (1, l1)):
                for k in range(len(lh)):
                    idx2[h][b2_of[h][(k, ch)], p] = grow_h[lh[k]]

        flat1 = src1.reshape(-1)
        x_exp = np.zeros((P, S1), dtype=xdt_np)
        valid = flat1 >= 0
        x_exp[:, valid] = xsT[:, flat1[valid]]

        slabs = []
        for gi, (half, blks) in enumerate(groups):
            idxs = np.empty((len(blks), P), np.int64)
            for j, (k, ch) in enumerate(blks):
                row = idx2[half][b2_of[half][(k, ch)]]
                idxs[j] = np.where(row >= 0, row, zero_row[half])
            flat = idxs.reshape(-1)
            assert flat.max() < 32768, flat.max()
            S = len(flat) // 16
            wrapped = flat.reshape(S, 16).T.astype(np.int16)  # [16, S]
            slabs.append(wrapped)
        idx_cat = np.concatenate(slabs, axis=1)
        idx_rep = np.tile(idx_cat, (8, 1))  # [128, sum S]

        dinv_loc = np.zeros(shard, np.float32)
        valid_pos = node_at[c] >= 0
        dinv_loc[valid_pos] = dinv[node_at[c][valid_pos]]
        dinv_fm = np.tile(dinv_loc[None, :], (64, 1)).astype(ml_dtypes.bfloat16)
        dinv_nm = dinv_loc.reshape(nch, P).T.astype(np.float32).copy()  # [128,nch]

        per_core.append(
            dict(x_exp=x_exp, idx=idx_rep, dinv_fm=dinv_fm, dinv_nm=dinv_nm)
        )

    struct = dict(
        N=N,
        shard=shard,
        nch=nch,
        h0ch=h0ch,
        h1ch=h1ch,
        H0=H0,
        H1=H1,
        blocks1=blocks1,
        nb1_g0=nb1_g0,
        groups=groups,
        group_segs=group_segs,
        blocks2=blocks2,
        S1=S1,
        node_at=node_at,
        idx_cols=per_core[0]["idx"].shape[1],
        K1g=K1g,
    )
    return struct, per_core, dinv


# ---------------------------------------------------------------------------
# Program builder
# ---------------------------------------------------------------------------

def _build(st, weights, n_passes=1):
    """Build the SPMD Bass program."""
    shard, nch = st["shard"], st["nch"]
    S1 = st["S1"]
    blocks1 = st["blocks1"]
    nb1_g0 = st["nb1_g0"]
    groups = st["groups"]
    group_segs = st["group_segs"]
    H0, H1 = st["H0"], st["H1"]
    h0ch, h1ch = st["h0ch"], st["h1ch"]
    bf16 = mybir.dt.bfloat16
    f32 = mybir.dt.float32
    f32r = mybir.dt.float32r
    tdt = f32r if (F32R and PE_REDUCE) else f32
    agdt = (mybir.dt.float8e4 if AG8 else bf16) if AGBF else tdt
    xdt = mybir.dt.float8e4 if X8 else bf16

    fb2 = float(np.asarray(weights["fc_b2"]).reshape(-1)[0])

    nc = bacc.Bacc(
        "TRN2",
        target_bir_lowering=False,
        debug=False,
        enable_asserts=False,
        num_devices=NCORES,
        num_swdge_queues=GQ,
    )

    x_exp_in = nc.dram_tensor("x_exp", [P, S1], xdt, kind="ExternalInput")
    idx_in = nc.dram_tensor(
        "idx2", [P, st["idx_cols"]], mybir.dt.int16, kind="ExternalInput"
    )
    dinv_fm_in = nc.dram_tensor("dinv_fm", [64, shard], bf16, kind="ExternalInput")
    dinv_nm_in = nc.dram_tensor("dinv_nm", [P, nch], f32, kind="ExternalInput")
    w1_in = nc.dram_tensor("w1", [P, 64], xdt, kind="ExternalInput")
    w2_in = nc.dram_tensor("w2", [64, 64], bf16, kind="ExternalInput")
    fw1_in = nc.dram_tensor("fw1", [64, 32], bf16, kind="ExternalInput")
    fw2_in = nc.dram_tensor("fw2", [32, 1], bf16, kind="ExternalInput")
    b1_in = nc.dram_tensor("b1c", [64, 1], f32, kind="ExternalInput")
    b2e_in = nc.dram_tensor("b2e", [P, 64], f32, kind="ExternalInput")
    fb1_in = nc.dram_tensor("fb1c", [32, 1], f32, kind="ExternalInput")
    y_out = nc.dram_tensor("y", [1, shard], f32, kind="ExternalOutput")

    with tile.TileContext(nc) as tc:
        with (
            tc.tile_pool(name="const", bufs=1) as constp,
            tc.tile_pool(name="big", bufs=1) as bigp,
            tc.tile_pool(name="xslab", bufs=5 if C1PSUM else 2) as xslabp,
            tc.tile_pool(name="gstage", bufs=3) as gstagep,
            tc.tile_pool(name="upc", bufs=2) as upcp,
            tc.tile_pool(name="psum", bufs=8, space="PSUM") as psump,
            tc.tile_pool(name="small", bufs=2) as smallp,
            tc.tile_pool(name="dram", bufs=1, space="DRAM") as dramp,
        ):
            # constants
            w1_sb = constp.tile([P, 64], xdt, name="w1_sb")
            nc.sync.dma_start(out=w1_sb[:], in_=w1_in.ap())
            w2_sb = constp.tile([64, 64], bf16, name="w2_sb")
            nc.sync.dma_start(out=w2_sb[:], in_=w2_in.ap())
            fw1_sb = constp.tile([64, 32], bf16, name="fw1_sb")
            nc.sync.dma_start(out=fw1_sb[:], in_=fw1_in.ap())
            fw2_sb = constp.tile([32, 1], bf16, name="fw2_sb")
            nc.sync.dma_start(out=fw2_sb[:], in_=fw2_in.ap())
            b1_sb = constp.tile([64, 1], f32, name="b1_sb")
            nc.sync.dma_start(out=b1_sb[:], in_=b1_in.ap())
            b2e_sb = constp.tile([P, 64], f32, name="b2e_sb")
            nc.sync.dma_start(out=b2e_sb[:], in_=b2e_in.ap())
            fb1_sb = constp.tile([32, 1], f32, name="fb1_sb")
            nc.sync.dma_start(out=fb1_sb[:], in_=fb1_in.ap())
            dinv_fm = constp.tile([64, shard], bf16, name="dinv_fm_sb")
            nc.sync.dma_start(out=dinv_fm[:], in_=dinv_fm_in.ap())
            dinv_nm = constp.tile([P, nch], f32, name="dinv_nm_sb")
            nc.sync.dma_start(out=dinv_nm[:], in_=dinv_nm_in.ap())
            ident = constp.tile([P, P], f32, name="ident")
            make_identity(nc, ident[:])
            idx_sb = constp.tile([P, st["idx_cols"]], mybir.dt.int16, name="idx_sb")
            if F32R and PE_REDUCE:
                ident_r = constp.tile([P, P], f32r, name="ident_r")
                nc.vector.tensor_copy(out=ident_r[:], in_=ident[:])
            else:
                ident_r = ident

            # DRAM tables
            t0h = dramp.tile([NCORES * H0, 64], agdt, name="t0h", addr_space="Shared")
            t1h = dramp.tile([NCORES * H1, 64], agdt, name="t1h", addr_space="Shared")
            if AGBF:
                t0 = dramp.tile([NCORES * H0, 64], tdt, name="t0")
                t1 = dramp.tile([NCORES * H1, 64], tdt, name="t1")
            else:
                t0, t1 = t0h, t1h
            ag0_in = dramp.tile([H0, 64], agdt, name="ag0i")
            ag1_in = dramp.tile([H1, 64], agdt, name="ag1i")

            h1s = bigp.tile([64, shard], bf16, name="h1s", tag="fm")
            acc1 = None
            if not C1PSUM:
                acc1 = bigp.tile([64, shard], f32, name="acc1", tag="acc1")

            # per-chunk conv1 round counts from blocks1
            _KT = {}
            for k, ch in blocks1:
                _KT[ch] = max(_KT.get(ch, 0), k + 1)

            # ------------- conv1 (per half group) + z2 + AllGather ----------
            def conv1_half(hg):
                b_lo = 0 if hg == 0 else nb1_g0
                b_hi = nb1_g0 if hg == 0 else len(blocks1)
                ch_lo = 0 if hg == 0 else h0ch
                nch_g = h0ch if hg == 0 else h1ch
                # PSUM regions: 4 chunks of [64,128] per bank
                nbank = -(-nch_g // 4)
                regs = []
                if C1PSUM:
                    for b in range(nbank):
                        rg = psump.tile(
                            [P, 512], f32, tag="ps", bufs=8, name=f"c1r_{hg}_{b}"
                        )
                        regs.append(rg)

                c_lo, c_hi = b_lo * P, b_hi * P
                n_slabs = -(-(c_hi - c_lo) // SLAB)
                # pre-enumerate matmul segments: (slab, i, r, bank, col)
                seg_list = []
                for si in range(n_slabs):
                    s0 = c_lo + si * SLAB
                    s1_ = min(c_hi, s0 + SLAB)
                    i = s0 // P
                    bend = s1_ // P
                    while i < bend:
                        k, ch = blocks1[i]
                        lch = ch - ch_lo
                        r = 1
                        while (
                            i + r < bend
                            and blocks1[i + r] == (k, ch + r)
                            and (lch + r) // 4 == lch // 4
                        ):
                            r += 1
                        seg_list.append([si, i, r, lch // 4, (lch % 4) * P])
                        i += r
                seen_b = set()
                last_of_b = {}
                flags = []
                for j, (si, i, r, b, col) in enumerate(seg_list):
                    st_f = b not in seen_b
                    seen_b.add(b)
                    last_of_b[b] = j
                    flags.append([st_f, False])
                for b, j in last_of_b.items():
                    flags[j][1] = True

                jseg = 0
                for si in range(n_slabs):
                    s0 = c_lo + si * SLAB
                    s1_ = min(c_hi, s0 + SLAB)
                    xsl = xslabp.tile([P, SLAB], xdt, tag="xsl", name=f"xsl_{hg}_{si}")
                    eng = nc.sync if (si % 2 == 0 or not ACTDMA) else nc.scalar
                    eng.dma_start(out=xsl[:, : s1_ - s0], in_=x_exp_in.ap()[:, s0:s1_])
                    if C1PSUM:
                        while jseg < len(seg_list) and seg_list[jseg][0] == si:
                            _, i, r, b, col = seg_list[jseg]
                            st_f, sp_f = flags[jseg]
                            nc.tensor.matmul(
                                regs[b][:64, col : col + r * P],
                                lhsT=w1_sb[:],
                                rhs=xsl[:, (i * P - s0) : (i * P - s0) + r * P],
                                start=st_f,
                                stop=sp_f,
                            )
                            jseg += 1
                    else:
                        for m0 in range(s0, s1_, 512):
                            m1 = min(s1_, m0 + 512)
                            pt = psump.tile(
                                [P, 512], f32, tag="ps", bufs=8, name=f"ps1_{hg}_{m0}"
                            )
                            nc.tensor.matmul(
                                pt[:64, : m1 - m0],
                                lhsT=w1_sb[:],
                                rhs=xsl[:, m0 - s0 : m1 - s0],
                                start=True,
                                stop=True,
                            )
                            b0, bend2 = m0 // P, m1 // P
                            i = b0
                            while i < bend2:
                                k, ch = blocks1[i]
                                r = 1
                                while i + r < bend2 and blocks1[i + r] == (k, ch + r):
                                    r += 1
                                if k == 0:
                                    nc.scalar.copy(
                                        out=acc1[:, ch * P : ch * P + r * P],
                                        in_=pt[:64, (i - b0) * P : (i - b0 + r) * P],
                                    )
                                else:
                                    nc.vector.tensor_add(
                                        acc1[:, ch * P : ch * P + r * P],
                                        acc1[:, ch * P : ch * P + r * P],
                                        pt[:64, (i - b0) * P : (i - b0 + r) * P],
                                    )
                                i += r

                # h1 = tanh(acc*dinv + b1), per bank readout
                cols0 = ch_lo * P
                for b in range(nbank):
                    w = min(512, (nch_g - b * 4) * P)
                    a0 = cols0 + b * 512
                    nc.vector.tensor_mul(
                        h1s[:, a0 : a0 + w],
                        regs[b][:64, :w] if C1PSUM else acc1[:, a0 : a0 + w],
                        dinv_fm[:, a0 : a0 + w],
                    )
                gw = nch_g * P
                nc.scalar.activation(
                    h1s[:, cols0 : cols0 + gw],
                    h1s[:, cols0 : cols0 + gw],
                    mybir.ActivationFunctionType.Tanh,
                    bias=b1_sb[:, :1],
                )

                # z2 (node-major bf16) for this half's chunks
                z2st = smallp.tile(
                    [P, nch_g * 64], agdt, tag="z2st", name=f"z2st_{hg}"
                )
                for j in range(nch_g):
                    ch = ch_lo + j
                    pz = psump.tile(
                        [P, 512], f32, tag="ps", bufs=8, name=f"pz_{hg}_{j}"
                    )
                    nc.tensor.matmul(
                        pz[:, :64],
                        lhsT=h1s[:, ch * P : (ch + 1) * P],
                        rhs=w2_sb[:],
                        start=True,
                        stop=True,
                    )
                    # z2 = (h1 @ W2) * dinv (per-node row scale, fused here)
                    if not ZSCALE:
                        nc.vector.tensor_scalar_mul(
                            z2st[:, j * 64 : (j + 1) * 64],
                            pz[:, :64],
                            dinv_nm[:, ch : ch + 1],
                        )
                    elif j % 2 == 0:
                        nc.scalar.activation(
                            z2st[:, j * 64 : (j + 1) * 64],
                            pz[:, :64],
                            mybir.ActivationFunctionType.Copy,
                            scale=dinv_nm[:, ch : ch + 1],
                        )
                    else:
                        nc.vector.tensor_scalar_mul(
                            z2st[:, j * 64 : (j + 1) * 64],
                            pz[:, :64],
                            dinv_nm[:, ch : ch + 1],
                        )

                ag_in = ag0_in if hg == 0 else ag1_in
                th = t0h if hg == 0 else t1h
                (nc.scalar if ACTDMA else nc.sync).dma_start(
                    out=ag_in[:].rearrange("(c p) f -> p c f", p=P),
                    in_=z2st[:].rearrange("p (c f) -> p c f", f=64),
                )
                nc.gpsimd.collective_compute(
                    "AllGather",
                    mybir.AluOpType.bypass,
                    replica_groups=[list(range(NCORES))],
                    ins=[ag_in.opt()],
                    outs=[th.opt()],
                )

            # upconvert one bf16 half-table to fp32, in pieces.
            # partition-major view: partition p holds table rows
            # [p*a_tot, (p+1)*a_tot) so each DMA is 128 contiguous runs.
            def upconvert(th, tf, rows, who, act_ok, gate=None):
                # act_ok=False keeps the ACT queue clear (so a later ag DMA
                # is not stuck behind pieces that wait on this AllGather).
                # gate: tiny DRAM AP whose write must precede this phase —
                # read one element into the staging tiles so the scheduler
                # cannot hoist these pieces ahead of the gate's producer.
                a_tot = rows // P
                PIECE = -(-a_tot // 8)
                for pi, a0 in enumerate(range(0, a_tot, PIECE)):
                    a1 = min(a_tot, a0 + PIECE)
                    w = (a1 - a0) * 64
                    ub = upcp.tile([P, PIECE * 64], agdt, tag="ub", name=f"ub_{who}_{pi}")
                    uf = upcp.tile([P, PIECE * 64], tdt, tag="uf", name=f"uf_{who}_{pi}")
                    eng = nc.scalar if (ACTDMA and act_ok and pi % 2 == 1) else nc.sync
                    if gate is not None and pi < 2:
                        eng.dma_start(out=ub[0:1, 0:1], in_=gate)
                        eng.dma_start(out=uf[0:1, 0:1].bitcast(agdt)[:, 0:1], in_=gate)
                    eng.dma_start(
                        out=ub[:, :w].rearrange("p (a f) -> p a f", f=64),
                        in_=th[:].rearrange("(p a) f -> p a f", p=P)[:, a0:a1, :],
                    )
                    if act_ok and pi % 2 == 1:
                        nc.scalar.copy(out=uf[:, :w], in_=ub[:, :w])
                    else:
                        nc.vector.tensor_copy(out=uf[:, :w], in_=ub[:, :w])
                    eng.dma_start(
                        out=tf[:].rearrange("(p a) f -> p a f", p=P)[:, a0:a1, :],
                        in_=uf[:, :w].rearrange("p (a f) -> p a f", f=64),
                    )

            # ------------- conv2 gathers + PE reduce ------------------------
            _gctr = [0]

            def conv2_half(sh, regs2):
                tab = t0 if sh == 0 else t1
                icol = _icol_of[sh]
                for gi, (half, blks) in enumerate(groups):
                    if half != sh:
                        continue
                    nb = len(blks)
                    nidx = nb * P
                    S = nidx // 16
                    stg = gstagep.tile(
                        [P, GBLK * 64], tdt, tag="stg", name=f"stg_{sh}_{gi}"
                    )
                    nc.gpsimd.dma_gather(
                        stg[:, : nb * 64].rearrange("p (b d) -> p b d", d=64),
                        tab[:],
                        idx_sb[:, icol : icol + S],
                        nidx,
                        nidx,
                        64,
                        queue_num=_gctr[0] % GQ,
                    )
                    _gctr[0] += 1
                    icol += S
                    for (i, bank, pcol, r, fstart, fstop) in group_segs[gi]:
                        if PE_REDUCE:
                            nc.tensor.matmul(
                                regs2[bank][:, pcol : pcol + r * 64],
                                lhsT=ident_r[:],
                                rhs=stg[:, i * 64 : (i + r) * 64],
                                start=fstart,
                                stop=fstop,
                            )
                        else:
                            k, ch = blks[i]
                            a0 = ch * 64
                            if fstart:
                                nc.scalar.copy(
                                    out=acc2[:, a0 : a0 + r * 64],
                                    in_=stg[:, i * 64 : (i + r) * 64],
                                )
                            else:
                                nc.vector.tensor_add(
                                    acc2[:, a0 : a0 + r * 64],
                                    acc2[:, a0 : a0 + r * 64],
                                    stg[:, i * 64 : (i + r) * 64],
                                )

            # precompute idx column offsets per source half
            _icol_of = {0: 0, 1: 0}
            icol = 0
            for gi, (half, blks) in enumerate(groups):
                if half == 1 and _icol_of[1] == 0:
                    _icol_of[1] = icol
                icol += len(blks) * P // 16

            # =================== emission order =============================
            conv1_half(0)
            # idx table is not needed until the conv2 gathers; load it in the
            # SP idle window so it does not delay the first conv1 slab
            nc.sync.dma_start(out=idx_sb[:], in_=idx_in.ap())
            conv1_half(1)
            if AGBF:
                upconvert(t0h, t0, NCORES * H0, "t0", act_ok=False, gate=ag1_in[0:1, 0:1])

            if PE_REDUCE:
                regs2 = [
                    psump.tile([P, 512], f32, tag="ps", bufs=8, name=f"c2r_{b}")
                    for b in range(-(-nch // 8))
                ]
                acc2 = None
            else:
                regs2 = None
                acc2 = bigp.tile([P, nch * 64], f32, name="acc2", tag="acc2")

            conv2_half(0, regs2)
            if AGBF:
                upconvert(t1h, t1, NCORES * H1, "t1", act_ok=True)
            conv2_half(1, regs2)

            # h2 = tanh(acc2*dinv_nm + b2) node-major, batched per PSUM bank
            h2 = bigp.tile([P, nch * 64], f32, name="h2", tag="h2")
            for b in range(-(-nch // 8)):
                rb = min(8, nch - b * 8)
                w = rb * 64
                a0 = b * 512
                if PE_REDUCE:
                    src_ap = regs2[b][:, :w]
                else:
                    src_ap = acc2[:, a0 : a0 + w]
                nc.vector.tensor_mul(
                    h2[:, a0 : a0 + w].rearrange("p (c f) -> p c f", f=64),
                    src_ap.rearrange("p (c f) -> p c f", f=64),
                    dinv_nm[:, b * 8 : b * 8 + rb, None].to_broadcast([P, rb, 64]),
                )
                nc.vector.tensor_add(
                    h2[:, a0 : a0 + w].rearrange("p (c f) -> p c f", f=64),
                    h2[:, a0 : a0 + w].rearrange("p (c f) -> p c f", f=64),
                    b2e_sb[:, None, :].to_broadcast([P, rb, 64]),
                )
            nc.scalar.activation(h2[:], h2[:], mybir.ActivationFunctionType.Tanh)

            # ------------- FC head -----------------------------------------
            h2fm = bigp.tile([64, shard], bf16, name="h2fm", tag="fm")
            for ch in range(nch):
                ptr = psump.tile([P, 512], f32, tag="ps", bufs=8, name=f"pst_{ch}")
                nc.tensor.transpose(
                    out=ptr[:64, :P],
                    in_=h2[:, ch * 64 : (ch + 1) * 64],
                    identity=ident[:],
                )
                if ch % 2 == 0:
                    nc.scalar.copy(out=h2fm[:, ch * P : (ch + 1) * P], in_=ptr[:64, :P])
                else:
                    nc.vector.tensor_copy(out=h2fm[:, ch * P : (ch + 1) * P], in_=ptr[:64, :P])

            h3 = bigp.tile([32, shard], bf16, name="h3", tag="h3")
            for m0 in range(0, shard, 512):
                m1 = min(shard, m0 + 512)
                pf = psump.tile([P, 512], f32, tag="ps", bufs=8, name=f"psf_{m0}")
                nc.tensor.matmul(
                    pf[:32, : m1 - m0], lhsT=fw1_sb[:], rhs=h2fm[:, m0:m1],
                    start=True, stop=True,
                )
                nc.scalar.activation(
                    h3[:, m0:m1],
                    pf[:32, : m1 - m0],
                    mybir.ActivationFunctionType.Tanh,
                    bias=fb1_sb[:, :1],
                )
            ysb = smallp.tile([1, shard], f32, tag="ysb", bufs=1, name="ysb")
            for m0 in range(0, shard, 512):
                m1 = min(shard, m0 + 512)
                pg = psump.tile([P, 512], f32, tag="ps", bufs=8, name=f"psg_{m0}")
                nc.tensor.matmul(
                    pg[:1, : m1 - m0], lhsT=fw2_sb[:], rhs=h3[:, m0:m1],
                    start=True, stop=True,
                )
                if (m0 // 512) % 2 == 0:
                    nc.scalar.activation(
                        ysb[:, m0:m1],
                        pg[:1, : m1 - m0],
                        mybir.ActivationFunctionType.Copy,
                        bias=fb2,
                    )
                else:
                    nc.vector.tensor_scalar_add(ysb[:, m0:m1], pg[:1, : m1 - m0], fb2)
            nc.sync.dma_start(out=y_out.ap(), in_=ysb[:])

    nc.compile()
    return nc


# ---------------------------------------------------------------------------
# Entry point
# ---------------------------------------------------------------------------

def _in_maps(st, per_core, weights):
    w1dt = (
        ml_dtypes.float8_e4m3fn
        if os.environ.get("GCN_X8", "0") == "1"
        else ml_dtypes.bfloat16
    )
    w1 = np.asarray(weights["conv_w1"], np.float32).astype(w1dt)
    w2 = np.asarray(weights["conv_w2"], np.float32).astype(ml_dtypes.bfloat16)
    fw1 = np.asarray(weights["fc_w1"], np.float32).astype(ml_dtypes.bfloat16)
    fw2 = np.asarray(weights["fc_w2"], np.float32).astype(ml_dtypes.bfloat16)
    b1 = np.asarray(weights["conv_b1"], np.float32).reshape(64, 1)
    b2e = np.tile(np.asarray(weights["conv_b2"], np.float32)[None, :], (P, 1))
    fb1 = np.asarray(weights["fc_b1"], np.float32).reshape(32, 1)
    maps = []
    for c in range(NCORES):
        pc = per_core[c]
        maps.append(
            {
                "x_exp": pc["x_exp"],
                "idx2": pc["idx"],
                "dinv_fm": pc["dinv_fm"],
                "dinv_nm": pc["dinv_nm"],
                "w1": np.ascontiguousarray(w1),
                "w2": np.ascontiguousarray(w2),
                "fw1": np.ascontiguousarray(fw1),
                "fw2": np.ascontiguousarray(fw2),
                "b1c": b1,
                "b2e": b2e,
                "fb1c": fb1,
            }
        )
    return maps


def kernel(**inputs):
    x = np.asarray(inputs["x"], np.float32)
    edge_index = np.asarray(inputs["edge_index"])
    weights = {
        k: np.asarray(inputs[k], np.float32)
        for k in (
            "conv_w1",
            "conv_b1",
            "conv_w2",
            "conv_b2",
            "fc_w1",
            "fc_b1",
            "fc_w2",
            "fc_b2",
        )
    }
    st, per_core, dinv = _preprocess(x, edge_index)
    nc = _build(st, weights, n_passes=1)
    maps = _in_maps(st, per_core, weights)
    res = None
    for attempt in range(3):
        try:
            res = bass_utils.run_bass_kernel_spmd(
                nc, maps, core_ids=list(range(NCORES))
            )
            break
        except Exception as e:  # device wedge: retry
            if attempt == 2:
                raise
            print(f"[kernel] run attempt {attempt} failed ({e}); retrying")
    N, shard = st["N"], st["shard"]
    node_at = st["node_at"]
    y = np.empty((N, 1), np.float32)
    for c in range(NCORES):
        yc = res.results[c]["y"].reshape(shard)
        valid = node_at[c] >= 0
        y[node_at[c][valid], 0] = yc[valid]
    return y
